# revision 1
# baseline (speedup 1.0000x reference)
"""Trainium2 Bass kernel for nn_ContrastLoss_Disentangle.

Contract: kernel(**inputs) takes the FULL (unsharded) inputs and returns the
same structure the reference returns: (loss_label, loss_norm, loss_triple)
as float32 scalars.

Pipeline (8 NeuronCores, data-parallel):
  host:    pose norms + normalization + [C*D, Np] transpose (poseFT)
  device1: per-core: nlp-row sumsq (ACT square+accum), raw nlp.pose dots
           (DVE mul + grouped reduce), and a [512, 1024] block of the
           pm gram matrix (PE, f32r fast path)
  host:    scores + BCE, pm assembly + stable argsort rank-select (furthest)
  device2: per-core dots of the gathered "hard positive" nlp rows
  host:    triplet loss assembly
"""

import os
import numpy as np

import concourse.bass as bass
import concourse.tile as tile
from concourse import bacc, mybir
from concourse.bass2jax import install_neuronx_cc_hook, partition_id_tensor, _bass_exec_p

C, NP, K, D = 8, 2048, 4, 256
NN = NP * K          # 8192
NCORES = 8
NPL = NP // NCORES   # 256 poses per core
NNL = NN // NCORES   # 1024 nlp rows per core
NT = NNL // 128      # 8 nlp tiles per category per core
CD = C * D           # 2048 contraction size

# pm block grid: 4 row-blocks x 2 col-blocks
PM_MI, PM_NJ = 4, 2
PM_M = NP // PM_MI   # 512 rows per core block
PM_N = NP // PM_NJ   # 1024 cols per core block

PM_MODE = os.environ.get("PM_MODE", "f32r")  # "f32r" | "bf16split" | "f32"

_runners = {}


def _build_dots_kernel(with_pm: bool, with_ssq: bool = True):
    """Per-core program. Inputs (per core):
      nlp   [C, NNL, D] f32   (raw nlp rows of this core; pose-major blocks)
      pose  [C, NPL, D] f32   (normalized pose rows matching this core's nlps)
      pm_l / pm_r             (poseFT column slices; only when with_pm)
    Outputs:
      ssq  [128, C*NT] f32    sumsq of nlp rows (col = (cat*2+pt)*4+k)
      rdot [128, C*NT] f32    dot(nlp_row, poseF[row//4]), same layout
      pmblk [PM_M, PM_N] f32  (only when with_pm)
    """
    nc = bacc.Bacc("TRN2", target_bir_lowering=False, debug=False,
                   num_devices=NCORES)
    nlp = nc.dram_tensor("nlp", [C, NNL, D], mybir.dt.float32,
                         kind="ExternalInput").ap()
    pose = nc.dram_tensor("pose", [C, NPL, D], mybir.dt.float32,
                          kind="ExternalInput").ap()
    if with_pm:
        if PM_MODE == "bf16split":
            pml_h = nc.dram_tensor("pml_h", [CD, PM_M], mybir.dt.bfloat16, kind="ExternalInput").ap()
            pml_l = nc.dram_tensor("pml_l", [CD, PM_M], mybir.dt.bfloat16, kind="ExternalInput").ap()
            pmr_h = nc.dram_tensor("pmr_h", [CD, PM_N], mybir.dt.bfloat16, kind="ExternalInput").ap()
            pmr_l = nc.dram_tensor("pmr_l", [CD, PM_N], mybir.dt.bfloat16, kind="ExternalInput").ap()
        else:
            pmdt = mybir.dt.float32r if PM_MODE == "f32r" else mybir.dt.float32
            pm_l = nc.dram_tensor("pm_l", [CD, PM_M], pmdt, kind="ExternalInput").ap()
            pm_r = nc.dram_tensor("pm_r", [CD, PM_N], pmdt, kind="ExternalInput").ap()
        pmblk = nc.dram_tensor("pmblk", [PM_M, PM_N], mybir.dt.float32,
                               kind="ExternalOutput").ap()
    if with_ssq:
        ssq = nc.dram_tensor("ssq", [128, C * NT], mybir.dt.float32,
                             kind="ExternalOutput").ap()
    rdot = nc.dram_tensor("rdot", [128, C * NT], mybir.dt.float32,
                          kind="ExternalOutput").ap()

    with tile.TileContext(nc) as tc:
        with tc.tile_pool(name="io", bufs=int(os.environ.get("BUFS_IO", 6))) as io, \
             tc.tile_pool(name="pose_p", bufs=3) as pose_p, \
             tc.tile_pool(name="scr", bufs=4) as scr, \
             tc.tile_pool(name="accum", bufs=1) as accum, \
             tc.tile_pool(name="matres", bufs=1) as matres, \
             tc.tile_pool(name="rhs_p", bufs=int(os.environ.get("BUFS_RHS", 6))) as rhs_p, \
             tc.tile_pool(name="ev", bufs=3) as ev, \
             tc.tile_pool(name="ps", bufs=1, space="PSUM") as ps:

            if with_ssq:
                ssq_t = accum.tile([128, C * NT], mybir.dt.float32, tag="ssq")
                nc.gpsimd.memset(ssq_t[:], 0.0)
            rdot_t = accum.tile([128, C * NT], mybir.dt.float32, tag="rdot")
            nc.gpsimd.memset(rdot_t[:], 0.0)

            # ---------- sub-programs -------------------------------------
            def a_iter(i):
                cat, pt = i // 2, i % 2
                col4 = (cat * (NPL // 128) + pt) * K
                po = pose_p.tile([128, D], mybir.dt.float32, tag="po",
                                 name=f"po{i}")
                nc.sync.dma_start(po[:], pose[cat, 128 * pt:128 * (pt + 1), :])
                x = io.tile([128, K * D], mybir.dt.float32, tag="x",
                            name=f"x{i}")
                nc.sync.dma_start(
                    x[:], nlp[cat, 512 * pt:512 * (pt + 1), :]
                    .rearrange("(p k) d -> p k d", k=K))
                full = po[:]
                rep = bass.AP(tensor=full.tensor, offset=full.offset,
                              ap=[list(full.ap[0]), [0, K], [1, D]])
                z = scr.tile([128, K * D], mybir.dt.float32, tag="z",
                             name=f"z{i}")
                nc.vector.tensor_tensor(
                    z[:].rearrange("p (k d) -> p k d", k=K),
                    x[:].rearrange("p (k d) -> p k d", k=K),
                    rep, op=mybir.AluOpType.mult)
                if with_ssq:
                    # dots reduce on DVE, sumsq on ACT
                    nc.vector.tensor_reduce(
                        rdot_t[:, col4:col4 + K],
                        z[:].rearrange("p (k d) -> p k d", k=K),
                        axis=mybir.AxisListType.X, op=mybir.AluOpType.add)
                    s1 = scr.tile([128, D], mybir.dt.float32, tag="s1",
                                  name=f"s1_{i}")
                    for k in range(K):
                        nc.scalar.activation(
                            s1[:], x[:, D * k:D * (k + 1)],
                            mybir.ActivationFunctionType.Square,
                            accum_out=ssq_t[:, col4 + k:col4 + k + 1])
                elif i % 2 == 0:
                    # no sumsq needed -> alternate the dots reduce between
                    # the idle ACT (copy+accum) and the DVE (grouped reduce)
                    s1 = scr.tile([128, D], mybir.dt.float32, tag="s1",
                                  name=f"s1_{i}")
                    for k in range(K):
                        nc.scalar.activation(
                            s1[:], z[:, D * k:D * (k + 1)],
                            mybir.ActivationFunctionType.Copy,
                            accum_out=rdot_t[:, col4 + k:col4 + k + 1])
                else:
                    nc.vector.tensor_reduce(
                        rdot_t[:, col4:col4 + K],
                        z[:].rearrange("p (k d) -> p k d", k=K),
                        axis=mybir.AxisListType.X, op=mybir.AluOpType.add)

            accs = {}

            def pm_chunk(s):
                n, k = s // 16, s % 16
                KT = CD // 128
                if k == 0:
                    accs[n] = [ps.tile([128, 512], mybir.dt.float32,
                                       name=f"acc{n}{m}", tag=f"acc{n}{m}")
                               for m in range(PM_MI)]
                rt = rhs_p.tile([128, 512], pm_rdt, tag="rt", name=f"rt{s}")
                nc.sync.dma_start(rt[:], pm_r[128 * k:128 * (k + 1),
                                              512 * n:512 * (n + 1)])
                for m in range(PM_MI):
                    nc.tensor.matmul(accs[n][m][:],
                                     lt[:, k, 128 * m:128 * (m + 1)], rt[:],
                                     start=(k == 0), stop=(k == KT - 1))
                if k == KT - 1:
                    for m in range(PM_MI):
                        o = ev.tile([128, 512], mybir.dt.float32, tag="ev",
                                    name=f"ev{n}{m}")
                        nc.scalar.copy(o[:], accs[n][m][:])
                        nc.sync.dma_start(
                            pmblk[128 * m:128 * (m + 1),
                                  512 * n:512 * (n + 1)], o[:])

            # ---------- emission order: interleave pm with dots ----------
            if with_pm:
                assert PM_MODE in ("f32r", "f32"), "bf16split path removed"
                pm_rdt = mybir.dt.float32r if PM_MODE == "f32r" else mybir.dt.float32
                KT = CD // 128
                _order = os.environ.get("K1_ORDER", "A")
                # unblock DVE/ACT before the big lhsT load hits the queues
                a_iter(0)
                a_iter(1)
                lt = matres.tile([128, KT, PM_M], pm_rdt, tag="lt")
                # split the big lhsT load into 4 chunks
                for kc in range(4):
                    nc.sync.dma_start(
                        lt[:, 4 * kc:4 * (kc + 1), :],
                        pm_l[512 * kc:512 * (kc + 1), :]
                        .rearrange("(k p) m -> p k m", p=128))
                for s in range(32):
                    pm_chunk(s)
                    if _order == "A":
                        if s % 2 == 0 and 2 + s // 2 < 16:
                            a_iter(2 + s // 2)
                    else:
                        if 2 + s < 16:
                            a_iter(2 + s)
            else:
                for i in range(16):
                    a_iter(i)

            if with_ssq:
                nc.sync.dma_start(ssq[:], ssq_t[:])
            nc.sync.dma_start(rdot[:], rdot_t[:])

    nc.finalize()
    return nc


def _make_runner(nc):
    """Reusable jitted SPMD runner (replicates bass2jax.run_bass_via_pjrt but
    caches the compiled executable across calls)."""
    import jax
    from jax.sharding import Mesh, PartitionSpec
    from jax.experimental.shard_map import shard_map

    install_neuronx_cc_hook()
    partition_name = nc.partition_id_tensor.name if nc.partition_id_tensor else None
    in_names, out_names, out_avals = [], [], []
    for alloc in nc.m.functions[0].allocations:
        if not isinstance(alloc, mybir.MemoryLocationSet):
            continue
        name = alloc.memorylocations[0].name
        if alloc.kind == "ExternalInput":
            if name != partition_name:
                in_names.append(name)
        elif alloc.kind == "ExternalOutput":
            out_names.append(name)
            out_avals.append(jax.core.ShapedArray(
                tuple(alloc.tensor_shape), mybir.dt.np(alloc.dtype)))
    n_params = len(in_names)
    all_in = in_names + out_names + ([partition_name] if partition_name else [])

    def _body(*args):
        operands = list(args)
        if partition_name is not None:
            operands.append(partition_id_tensor())
        outs = _bass_exec_p.bind(
            *operands, out_avals=tuple(out_avals), in_names=tuple(all_in),
            out_names=tuple(out_names), lowering_input_output_aliases=(),
            sim_require_finite=False, sim_require_nnan=False, nc=nc)
        return tuple(outs)

    devices = jax.devices()[:NCORES]
    mesh = Mesh(np.asarray(devices), ("core",))
    donate = tuple(range(n_params, n_params + len(out_names)))
    sharded = jax.jit(
        shard_map(_body, mesh=mesh,
                  in_specs=(PartitionSpec("core"),) * (n_params + len(out_names)),
                  out_specs=(PartitionSpec("core"),) * len(out_names),
                  check_rep=False),
        donate_argnums=donate, keep_unused=True)

    def run(in_maps):
        concat_in = [np.concatenate([np.asarray(m[name]) for m in in_maps], axis=0)
                     for name in in_names]
        zeros = [np.zeros((NCORES * a.shape[0], *a.shape[1:]), a.dtype)
                 for a in out_avals]
        out_arrs = sharded(*concat_in, *zeros)
        return [
            {name: np.asarray(out_arrs[i]).reshape(NCORES, *out_avals[i].shape)[c]
             for i, name in enumerate(out_names)}
            for c in range(NCORES)
        ]

    return run


def _get_runner(key):
    if key not in _runners:
        if key == "k1":
            _runners[key] = _make_runner(_build_dots_kernel(with_pm=True))
        else:
            _runners[key] = _make_runner(
                _build_dots_kernel(with_pm=False, with_ssq=False))
    return _runners[key]


def _col_to_rows(a):
    """[8 cores][128, C*NT] device output -> [C, NN] (global nlp rows).

    column = (cat*2 + pt)*K + k ; partition p -> nlp row 512*pt + 4*p + k
    within the core shard (pose-major layout)."""
    out = np.empty((C, NN), np.float32)
    for c in range(NCORES):
        blk = a[c].reshape(128, C, NPL // 128, K)      # [p, cat, pt, k]
        out[:, c * NNL:(c + 1) * NNL] = (
            blk.transpose(1, 2, 0, 3).reshape(C, NNL))
    return out


def _kernel_host_fallback(inputs):
    """Pure-numpy reference replication, used only if the index tensors do
    not have the canonical arange structure the device layout relies on."""
    nlp = np.asarray(inputs["nlp_features"], np.float32)
    pose = np.asarray(inputs["pose_features"], np.float32)
    nlab = np.asarray(inputs["nlp_label"]).astype(np.int64)
    n2p = np.asarray(inputs["nlpid2poseid"]).astype(np.int64)
    p2n = np.asarray(inputs["pose2nlpid"]).astype(np.int64)
    cat = np.asarray(inputs["categories"], np.float32)
    ri = np.asarray(inputs["rand_index"]).astype(np.int64)
    Np, Nn = pose.shape[1], nlp.shape[1]
    norm_p = np.sqrt(np.einsum("cpd,cpd->cp", pose, pose, dtype=np.float32))
    norm_n = np.sqrt(np.einsum("cnd,cnd->cn", nlp, nlp, dtype=np.float32))
    poseF = pose / norm_p[:, :, None]
    nlpF = nlp / norm_n[:, :, None]
    loss_norm = np.float32(np.float32(norm_p.mean()) + np.float32(norm_n.mean()))
    dots = np.einsum("cnd,cnd->cn", nlpF, poseF[:, n2p]).astype(np.float32)
    scores = np.einsum("cn,nc->n", dots, cat).astype(np.float32)
    p = (1.0 / (1.0 + np.exp(-scores))).astype(np.float32)
    lblf = nlab.astype(np.float32)
    loss_label = np.float32(
        np.mean(-(np.log(p) * lblf + np.log(1.0 - p) * (1.0 - lblf))))
    pf = np.ascontiguousarray(poseF.transpose(0, 2, 1).reshape(-1, Np))
    pm = (pf.T @ pf).astype(np.float32)
    ar = np.arange(Np)
    pm[ar, ar] = 1.0
    order = np.argsort(pm, axis=1, kind="stable")
    furthest = order[ar, ri]
    sg = scores[p2n]
    lg = nlab[p2n]
    maxp = np.maximum(np.max(np.where(lg == 0, sg, -np.inf), axis=1), -1.0)
    minp = np.minimum(np.min(np.where(lg == 1, sg, np.inf), axis=1), 1.0)
    nids = p2n[furthest]
    cd = np.einsum("cpkd,cpd->cpk", nlpF[:, nids], poseF)
    cur = np.einsum("cpk,pkc->pk", cd, cat[nids]).astype(np.float32)
    lcur = nlab[nids]
    maxcur = np.max(np.where(lcur == 1, cur, -np.inf), axis=1)
    maxp = np.maximum(maxp, maxcur)
    found = ~((maxp == -1.0) | (minp == 1.0))
    lt = np.where(found, maxp - minp + 2.0, 0.0).astype(np.float32)
    nf = int(np.sum(~found))
    loss_triple = (np.float32(0.0) if nf == Nn else
                   np.float32(lt.sum(dtype=np.float32) / np.float32(Nn - nf)))
    return (np.float32(loss_label), loss_norm, loss_triple)


def kernel(**inputs):
    nlp = np.ascontiguousarray(inputs["nlp_features"], np.float32)      # [C, NN, D]
    pose = np.ascontiguousarray(inputs["pose_features"], np.float32)    # [C, NP, D]
    nlab = np.asarray(inputs["nlp_label"]).astype(np.int64)
    cat = np.ascontiguousarray(inputs["categories"], np.float32)        # [NN, C]
    ri = np.asarray(inputs["rand_index"]).astype(np.int64)

    n2p = np.asarray(inputs["nlpid2poseid"]).astype(np.int64)
    p2n = np.asarray(inputs["pose2nlpid"]).astype(np.int64)
    if (not np.array_equal(n2p, np.arange(NN) // K)
            or not np.array_equal(p2n, np.arange(NN).reshape(NP, K))):
        return _kernel_host_fallback(inputs)

    # ---- host: pose normalization + poseFT ------------------------------
    norm_p = np.sqrt(np.einsum("cpd,cpd->cp", pose, pose, dtype=np.float32,
                               optimize=True)).astype(np.float32)       # [C, NP]
    poseF = pose / norm_p[:, :, None]
    poseFT = np.ascontiguousarray(
        poseF.transpose(0, 2, 1).reshape(CD, NP))                       # [CD, NP]

    # ---- device kernel 1 -------------------------------------------------
    run1 = _get_runner("k1")
    in_maps = []
    for c in range(NCORES):
        i, j = c // PM_NJ, c % PM_NJ
        m = {
            "nlp": nlp[:, c * NNL:(c + 1) * NNL, :],
            "pose": poseF[:, c * NPL:(c + 1) * NPL, :],
            "pm_l": np.ascontiguousarray(poseFT[:, i * PM_M:(i + 1) * PM_M]),
            "pm_r": np.ascontiguousarray(poseFT[:, j * PM_N:(j + 1) * PM_N]),
        }
        in_maps.append(m)
    res1 = run1(in_maps)

    ssq_n = _col_to_rows([r["ssq"] for r in res1])                      # [C, NN]
    rdot = _col_to_rows([r["rdot"] for r in res1])                      # [C, NN]
    pm = np.empty((NP, NP), np.float32)
    for c in range(NCORES):
        i, j = c // PM_NJ, c % PM_NJ
        pm[i * PM_M:(i + 1) * PM_M, j * PM_N:(j + 1) * PM_N] = res1[c]["pmblk"]

    # ---- host: norms / scores / BCE -------------------------------------
    norm_n = np.sqrt(ssq_n)                                             # [C, NN]
    loss_norm = np.float32(np.float32(norm_p.mean()) + np.float32(norm_n.mean()))

    dots = (rdot / norm_n).astype(np.float32)                           # [C, NN]
    scores = np.einsum("cn,nc->n", dots, cat).astype(np.float32)
    p = (1.0 / (1.0 + np.exp(-scores))).astype(np.float32)
    lblf = nlab.astype(np.float32)
    loss_label = np.float32(
        np.mean(-(np.log(p) * lblf + np.log(1.0 - p) * (1.0 - lblf))))

    # ---- host: furthest selection ---------------------------------------
    ar = np.arange(NP)
    pm[ar, ar] = 1.0
    order = np.argsort(pm, axis=1, kind="stable")
    furthest = order[ar, ri]                                            # [NP]

    sg = scores.reshape(NP, K)
    lg = nlab.reshape(NP, K)
    maxp = np.maximum(np.max(np.where(lg == 0, sg, -np.inf), axis=1), -1.0)
    minp = np.minimum(np.min(np.where(lg == 1, sg, np.inf), axis=1), 1.0)

    nids = (furthest[:, None] * K + np.arange(K)).reshape(-1)           # [NN]

    # ---- device kernel 2: dots of gathered hard-positive rows ------------
    run2 = _get_runner("k2")
    gnlp = nlp[:, nids, :]                                              # [C, NN, D]
    in_maps2 = []
    for c in range(NCORES):
        in_maps2.append({
            "nlp": gnlp[:, c * NNL:(c + 1) * NNL, :],
            "pose": poseF[:, c * NPL:(c + 1) * NPL, :],
        })
    res2 = run2(in_maps2)
    crdot = _col_to_rows([r["rdot"] for r in res2])                     # [C, NN]

    cur_dots = (crdot / norm_n[:, nids]).astype(np.float32)             # [C, NN]
    cur = np.einsum("cn,nc->n", cur_dots, cat[nids]).astype(np.float32)
    cur = cur.reshape(NP, K)
    lcur = nlab[nids].reshape(NP, K)
    maxcur = np.max(np.where(lcur == 1, cur, -np.inf), axis=1)
    maxp = np.maximum(maxp, maxcur)
    found = ~((maxp == -1.0) | (minp == 1.0))
    lt = np.where(found, maxp - minp + 2.0, 0.0).astype(np.float32)
    not_find = int(np.sum(~found))
    if not_find == NN:
        loss_triple = np.float32(0.0)
    else:
        loss_triple = np.float32(lt.sum(dtype=np.float32) / np.float32(NN - not_find))

    return (np.float32(loss_label), np.float32(loss_norm), np.float32(loss_triple))



# revision 4
# speedup vs baseline: 1.8232x; 1.8232x over previous
"""Trainium2 Bass kernel for nn_ContrastLoss_Disentangle.

Contract: kernel(**inputs) takes the FULL (unsharded) inputs and returns
(loss_label, loss_norm, loss_triple) as float32 scalars.

Pipeline (8 NeuronCores, data-parallel over pose/nlp rows):
  host:    norms (pose+nlp), normalization, X = nlpF * categories / norm
           (bf16, cd-partition interleaved layout), poseFT fp8 (x16) for the
           gram matrix, poseFT bf16 interleaved for the dot products
  device1: per-core: scores for the core's 1024 nlp rows (DVE bf16 mult +
           PE ones-matmul partition reduction) and a [512, 1024] block of
           the pose gram matrix (PE, fp8 DoubleRow), emitted as f16
  host:    BCE, pm assembly + stable argsort rank-select (furthest), gather
           of the hard-positive X columns
  device2: per-core: cur dots of the gathered X columns (same structure)
  host:    triplet loss assembly
"""

import os
import numpy as np
import ml_dtypes

import concourse.bass as bass
import concourse.tile as tile
from concourse import bacc, mybir
from concourse.bass2jax import install_neuronx_cc_hook, partition_id_tensor, _bass_exec_p

C, NP, K, D = 8, 2048, 4, 256
NN = NP * K          # 8192
NCORES = 8
NPL = NP // NCORES   # 256 poses per core
NNL = NN * 1 // NCORES  # 1024 nlp rows per core
CD = C * D           # 2048 contraction size
KC = CD // 128       # 16 cd chunks of 128 partitions

# pm block grid: 4 row-blocks x 2 col-blocks
PM_MI, PM_NJ = 4, 2
PM_M = NP // PM_MI   # 512 rows per core block
PM_N = NP // PM_NJ   # 1024 cols per core block
KT2 = CD // 256      # 8 DoubleRow contraction chunks

PM_SCALE = 16.0      # fp8 pre-scale (argsort is scale-invariant)

BF16 = ml_dtypes.bfloat16
FP8 = ml_dtypes.float8_e4m3

_runners = {}


def _build_kernel(with_pm: bool):
    """Per-core program. Inputs (per core):
      xt  [128, KC, K, NPL] bf16  X columns, cd-interleaved, (k, pose)-major
      po  [128, KC, NPL]    bf16  poseFT columns of this core, cd-interleaved
      pml [CD, PM_M] fp8, pmr [CD, PM_N] fp8  (only when with_pm)
    Outputs:
      sc  [1, K*NPL] f32   scores, col = k*NPL + p_local
      pmblk [PM_M, PM_N] f16  (only when with_pm)
    """
    f8 = mybir.dt.float8e4
    bf = mybir.dt.bfloat16
    f16 = mybir.dt.float16
    f32 = mybir.dt.float32
    nc = bacc.Bacc("TRN2", target_bir_lowering=False, debug=False,
                   num_devices=NCORES)
    xt = nc.dram_tensor("xt", [128, KC, K, NPL], bf, kind="ExternalInput").ap()
    po = nc.dram_tensor("po", [128, KC, NPL], bf, kind="ExternalInput").ap()
    if with_pm:
        pml = nc.dram_tensor("pml", [CD, PM_M], f8, kind="ExternalInput").ap()
        pmr = nc.dram_tensor("pmr", [CD, PM_N], f8, kind="ExternalInput").ap()
        pmblk = nc.dram_tensor("pmblk", [PM_M, PM_N], f16,
                               kind="ExternalOutput").ap()
    sc = nc.dram_tensor("sc", [1, K * NPL], f32, kind="ExternalOutput").ap()

    DR = mybir.MatmulPerfMode.DoubleRow
    with tile.TileContext(nc) as tc:
        with tc.tile_pool(name="xin", bufs=3) as xin, \
             tc.tile_pool(name="cst", bufs=1) as cst, \
             tc.tile_pool(name="zp", bufs=3) as zp, \
             tc.tile_pool(name="rhs_p", bufs=4) as rhs_p, \
             tc.tile_pool(name="ev", bufs=3) as ev, \
             tc.tile_pool(name="ps", bufs=1, space="PSUM") as ps, \
             tc.tile_pool(name="ps_s", bufs=1, space="PSUM") as ps_s:

            ones = cst.tile([128, 1], bf, tag="ones")
            nc.gpsimd.memset(ones[:], 1.0)
            po_t = cst.tile([128, KC, NPL], bf, tag="po")
            nc.sync.dma_start(po_t[:], po)

            acc_s = [ps_s.tile([1, 512], f32, tag=f"accs{h}", name=f"accs{h}")
                     for h in range(2)]

            # ---------- scores sub-program (one cd chunk) -----------------
            XCH = 4  # kc chunks per xt DMA
            xt_t = cst.tile([128, KC, K, NPL], bf, tag="xt")

            def x_load(ci):
                nc.sync.dma_start(xt_t[:, XCH * ci:XCH * (ci + 1)],
                                  xt[:, XCH * ci:XCH * (ci + 1)])

            def s_iter(kc):
                z = zp.tile([128, K, NPL], bf, tag="z", name=f"z{kc}")
                full = po_t[:, kc]
                rep = bass.AP(tensor=full.tensor, offset=full.offset,
                              ap=[list(full.ap[0]), [0, K], list(full.ap[1])])
                nc.vector.tensor_tensor(z[:], xt_t[:, kc], rep,
                                        op=mybir.AluOpType.mult)
                zf = z[:].rearrange("p k n -> p (k n)")
                for h in range(2):
                    nc.tensor.matmul(acc_s[h][:], ones[:],
                                     zf[:, 512 * h:512 * (h + 1)],
                                     start=(kc == 0), stop=(kc == KC - 1),
                                     skip_group_check=True)

            def s_flush():
                o = ev.tile([1, K * NPL], f32, tag="so")
                for h in range(2):
                    nc.scalar.copy(o[:, 512 * h:512 * (h + 1)], acc_s[h][:])
                nc.sync.dma_start(sc[:], o[:])

            # ---------- pm sub-program ------------------------------------
            if with_pm:
                lt = cst.tile([128, KT2, 2, PM_M], f8, tag="lt")
                accs = [ps.tile([128, 4, 512], f32, tag="acc", name="acc")]

                def lt_load(half):
                    nc.sync.dma_start(
                        lt[:, 4 * half:4 * (half + 1)],
                        pml[1024 * half:1024 * (half + 1), :]
                        .rearrange("(kc t p) m -> p kc t m", p=128, t=2))

                def pm_chunk(s):
                    n, kc = s // KT2, s % KT2
                    rt = rhs_p.tile([128, 2, 512], f8, tag="rt", name=f"rt{s}")
                    nc.sync.dma_start(
                        rt[:], pmr[256 * kc:256 * (kc + 1),
                                   512 * n:512 * (n + 1)]
                        .rearrange("(t p) m -> p t m", p=128))
                    acc = accs[0]
                    for m in range(PM_MI):
                        nc.tensor.matmul(acc[:, m][:],
                                         lt[:, kc, :, 128 * m:128 * (m + 1)],
                                         rt[:], start=(kc == 0),
                                         stop=(kc == KT2 - 1),
                                         perf_mode=DR, skip_group_check=True)
                    if kc == KT2 - 1:
                        for m in range(PM_MI):
                            o = ev.tile([128, 512], f16, tag="ev",
                                        name=f"ev{n}{m}")
                            nc.scalar.copy(o[:], acc[:, m][:])
                            nc.sync.dma_start(
                                pmblk[128 * m:128 * (m + 1),
                                      512 * n:512 * (n + 1)], o[:])

            # ---------- emission order ------------------------------------
            if with_pm:
                x_load(0)
                s_iter(0)
                lt_load(0)
                lt_load(1)
                s_iter(1)
                # interleave: 16 pm chunks, 14 remaining s_iters
                xi = 2
                for s in range(2 * KT2):
                    pm_chunk(s)
                    if s % 4 == 0 and xi // XCH + 1 <= 3 and xi % XCH == 2:
                        pass
                    if xi < KC:
                        if xi % XCH == 0:
                            x_load(xi // XCH)
                        s_iter(xi)
                        xi += 1
                while xi < KC:
                    if xi % XCH == 0:
                        x_load(xi // XCH)
                    s_iter(xi)
                    xi += 1
                s_flush()
            else:
                for ci in range(KC // XCH):
                    x_load(ci)
                    for kc in range(XCH * ci, XCH * (ci + 1)):
                        s_iter(kc)
                s_flush()

    nc.finalize()
    return nc


def _make_runner(nc):
    """Reusable jitted SPMD runner (replicates bass2jax.run_bass_via_pjrt but
    caches the compiled executable across calls)."""
    import jax
    from jax.sharding import Mesh, PartitionSpec
    from jax.experimental.shard_map import shard_map

    install_neuronx_cc_hook()
    partition_name = nc.partition_id_tensor.name if nc.partition_id_tensor else None
    in_names, out_names, out_avals = [], [], []
    for alloc in nc.m.functions[0].allocations:
        if not isinstance(alloc, mybir.MemoryLocationSet):
            continue
        name = alloc.memorylocations[0].name
        if alloc.kind == "ExternalInput":
            if name != partition_name:
                in_names.append(name)
        elif alloc.kind == "ExternalOutput":
            out_names.append(name)
            out_avals.append(jax.core.ShapedArray(
                tuple(alloc.tensor_shape), mybir.dt.np(alloc.dtype)))
    n_params = len(in_names)
    all_in = in_names + out_names + ([partition_name] if partition_name else [])

    def _body(*args):
        operands = list(args)
        if partition_name is not None:
            operands.append(partition_id_tensor())
        outs = _bass_exec_p.bind(
            *operands, out_avals=tuple(out_avals), in_names=tuple(all_in),
            out_names=tuple(out_names), lowering_input_output_aliases=(),
            sim_require_finite=False, sim_require_nnan=False, nc=nc)
        return tuple(outs)

    devices = jax.devices()[:NCORES]
    mesh = Mesh(np.asarray(devices), ("core",))
    donate = tuple(range(n_params, n_params + len(out_names)))
    sharded = jax.jit(
        shard_map(_body, mesh=mesh,
                  in_specs=(PartitionSpec("core"),) * (n_params + len(out_names)),
                  out_specs=(PartitionSpec("core"),) * len(out_names),
                  check_rep=False),
        donate_argnums=donate, keep_unused=True)

    def run(in_maps):
        concat_in = [np.concatenate([np.asarray(m[name]) for m in in_maps], axis=0)
                     for name in in_names]
        zeros = [np.zeros((NCORES * a.shape[0], *a.shape[1:]), a.dtype)
                 for a in out_avals]
        out_arrs = sharded(*concat_in, *zeros)
        return [
            {name: np.asarray(out_arrs[i]).reshape(NCORES, *out_avals[i].shape)[c]
             for i, name in enumerate(out_names)}
            for c in range(NCORES)
        ]

    return run


def _get_runner(key):
    if key not in _runners:
        _runners[key] = _make_runner(_build_kernel(with_pm=(key == "k1")))
    return _runners[key]


def _interleave_cols(A):
    """[CD, cols] -> [128, KC, cols]: partition p holds cd rows kc*128+p."""
    return np.ascontiguousarray(
        A.reshape(KC, 128, A.shape[1]).transpose(1, 0, 2))


def _x_dev(Xt16, cols):
    """Columns `cols` (global nlp ids, (k, pose)-major order per core) of the
    bf16 [CD, NN] X^T matrix -> [128, KC, K, NPL] device layout."""
    return _interleave_cols(Xt16[:, cols]).reshape(128, KC, K, NPL)


def _scores_from_dev(res, name="sc"):
    """Per-core [1, K*NPL] (col = k*NPL + p_local) -> [NN] global scores."""
    out = np.empty((NCORES, NPL, K), np.float32)
    for c in range(NCORES):
        out[c] = res[c][name].reshape(K, NPL).T
    return out.reshape(NN)


def _kernel_host_fallback(inputs):
    """Pure-numpy reference replication, used only if the index tensors do
    not have the canonical arange structure the device layout relies on."""
    nlp = np.asarray(inputs["nlp_features"], np.float32)
    pose = np.asarray(inputs["pose_features"], np.float32)
    nlab = np.asarray(inputs["nlp_label"]).astype(np.int64)
    n2p = np.asarray(inputs["nlpid2poseid"]).astype(np.int64)
    p2n = np.asarray(inputs["pose2nlpid"]).astype(np.int64)
    cat = np.asarray(inputs["categories"], np.float32)
    ri = np.asarray(inputs["rand_index"]).astype(np.int64)
    Np, Nn = pose.shape[1], nlp.shape[1]
    norm_p = np.sqrt(np.einsum("cpd,cpd->cp", pose, pose, dtype=np.float32))
    norm_n = np.sqrt(np.einsum("cnd,cnd->cn", nlp, nlp, dtype=np.float32))
    poseF = pose / norm_p[:, :, None]
    nlpF = nlp / norm_n[:, :, None]
    loss_norm = np.float32(np.float32(norm_p.mean()) + np.float32(norm_n.mean()))
    dots = np.einsum("cnd,cnd->cn", nlpF, poseF[:, n2p]).astype(np.float32)
    scores = np.einsum("cn,nc->n", dots, cat).astype(np.float32)
    p = (1.0 / (1.0 + np.exp(-scores))).astype(np.float32)
    lblf = nlab.astype(np.float32)
    loss_label = np.float32(
        np.mean(-(np.log(p) * lblf + np.log(1.0 - p) * (1.0 - lblf))))
    pf = np.ascontiguousarray(poseF.transpose(0, 2, 1).reshape(-1, Np))
    pm = (pf.T @ pf).astype(np.float32)
    ar = np.arange(Np)
    pm[ar, ar] = 1.0
    order = np.argsort(pm, axis=1, kind="stable")
    furthest = order[ar, ri]
    sg = scores[p2n]
    lg = nlab[p2n]
    maxp = np.maximum(np.max(np.where(lg == 0, sg, -np.inf), axis=1), -1.0)
    minp = np.minimum(np.min(np.where(lg == 1, sg, np.inf), axis=1), 1.0)
    nids = p2n[furthest]
    cd = np.einsum("cpkd,cpd->cpk", nlpF[:, nids], poseF)
    cur = np.einsum("cpk,pkc->pk", cd, cat[nids]).astype(np.float32)
    lcur = nlab[nids]
    maxcur = np.max(np.where(lcur == 1, cur, -np.inf), axis=1)
    maxp = np.maximum(maxp, maxcur)
    found = ~((maxp == -1.0) | (minp == 1.0))
    lt = np.where(found, maxp - minp + 2.0, 0.0).astype(np.float32)
    nf = int(np.sum(~found))
    loss_triple = (np.float32(0.0) if nf == Nn else
                   np.float32(lt.sum(dtype=np.float32) / np.float32(Nn - nf)))
    return (np.float32(loss_label), loss_norm, loss_triple)


def kernel(**inputs):
    nlp = np.ascontiguousarray(inputs["nlp_features"], np.float32)      # [C, NN, D]
    pose = np.ascontiguousarray(inputs["pose_features"], np.float32)    # [C, NP, D]
    nlab = np.asarray(inputs["nlp_label"]).astype(np.int64)
    cat = np.ascontiguousarray(inputs["categories"], np.float32)        # [NN, C]
    ri = np.asarray(inputs["rand_index"]).astype(np.int64)

    n2p = np.asarray(inputs["nlpid2poseid"]).astype(np.int64)
    p2n = np.asarray(inputs["pose2nlpid"]).astype(np.int64)
    if (not np.array_equal(n2p, np.arange(NN) // K)
            or not np.array_equal(p2n, np.arange(NN).reshape(NP, K))):
        return _kernel_host_fallback(inputs)

    # ---- host: norms, X^T (bf16), poseFT (bf16 + fp8) --------------------
    norm_p = np.sqrt(np.einsum("cpd,cpd->cp", pose, pose, dtype=np.float32,
                               optimize=True)).astype(np.float32)       # [C, NP]
    norm_n = np.sqrt(np.einsum("cnd,cnd->cn", nlp, nlp, dtype=np.float32,
                               optimize=True)).astype(np.float32)       # [C, NN]
    loss_norm = np.float32(np.float32(norm_p.mean()) + np.float32(norm_n.mean()))

    poseF = pose / norm_p[:, :, None]
    poseFT = np.ascontiguousarray(
        poseF.transpose(0, 2, 1).reshape(CD, NP))                       # [CD, NP]
    P8 = (poseFT * PM_SCALE).astype(FP8)                                # [CD, NP]

    scale = (cat.T / norm_n).astype(np.float32)                         # [C, NN]
    Xt16 = np.ascontiguousarray(
        (nlp * scale[:, :, None]).transpose(0, 2, 1).reshape(CD, NN)
    ).astype(BF16)                                                      # [CD, NN]

    # (k, pose)-major column order within each core's 1024 nlp columns
    base_cols = (np.arange(NPL)[None, :] * K + np.arange(K)[:, None]).reshape(-1)

    # ---- device kernel 1: scores + pm blocks -----------------------------
    run1 = _get_runner("k1")
    in_maps = []
    for c in range(NCORES):
        i, j = c // PM_NJ, c % PM_NJ
        in_maps.append({
            "xt": _x_dev(Xt16, c * NNL + base_cols),
            "po": _interleave_cols(
                np.ascontiguousarray(poseFT[:, c * NPL:(c + 1) * NPL])
                .astype(BF16)),
            "pml": np.ascontiguousarray(P8[:, i * PM_M:(i + 1) * PM_M]),
            "pmr": np.ascontiguousarray(P8[:, j * PM_N:(j + 1) * PM_N]),
        })
    res1 = run1(in_maps)

    scores = _scores_from_dev(res1)                                     # [NN]
    pm = np.empty((NP, NP), np.float32)
    for c in range(NCORES):
        i, j = c // PM_NJ, c % PM_NJ
        pm[i * PM_M:(i + 1) * PM_M, j * PM_N:(j + 1) * PM_N] = \
            res1[c]["pmblk"].astype(np.float32)

    # ---- host: BCE -------------------------------------------------------
    p = (1.0 / (1.0 + np.exp(-scores))).astype(np.float32)
    lblf = nlab.astype(np.float32)
    loss_label = np.float32(
        np.mean(-(np.log(p) * lblf + np.log(1.0 - p) * (1.0 - lblf))))

    # ---- host: furthest selection (pm is PM_SCALE^2 * gram) --------------
    ar = np.arange(NP)
    pm[ar, ar] = PM_SCALE * PM_SCALE
    order = np.argsort(pm, axis=1, kind="stable")
    furthest = order[ar, ri]                                            # [NP]

    sg = scores.reshape(NP, K)
    lg = nlab.reshape(NP, K)
    maxp = np.maximum(np.max(np.where(lg == 0, sg, -np.inf), axis=1), -1.0)
    minp = np.minimum(np.min(np.where(lg == 1, sg, np.inf), axis=1), 1.0)

    # ---- device kernel 2: dots of gathered hard-positive columns ---------
    run2 = _get_runner("k2")
    in_maps2 = []
    for c in range(NCORES):
        # col j = k*NPL + p_local -> global nlp id 4*furthest[p] + k
        fth = furthest[c * NPL:(c + 1) * NPL]
        cols = (np.arange(K)[:, None] + fth[None, :] * K).reshape(-1)
        in_maps2.append({
            "xt": _x_dev(Xt16, cols),
            "po": in_maps[c]["po"],
        })
    res2 = run2(in_maps2)
    cur = _scores_from_dev(res2).reshape(NP, K)

    nids = (furthest[:, None] * K + np.arange(K)).reshape(-1)           # [NN]
    lcur = nlab[nids].reshape(NP, K)
    maxcur = np.max(np.where(lcur == 1, cur, -np.inf), axis=1)
    maxp = np.maximum(maxp, maxcur)
    found = ~((maxp == -1.0) | (minp == 1.0))
    lt = np.where(found, maxp - minp + 2.0, 0.0).astype(np.float32)
    not_find = int(np.sum(~found))
    if not_find == NN:
        loss_triple = np.float32(0.0)
    else:
        loss_triple = np.float32(lt.sum(dtype=np.float32) / np.float32(NN - not_find))

    return (np.float32(loss_label), np.float32(loss_norm), np.float32(loss_triple))


# revision 7
# speedup vs baseline: 2.0258x; 1.1111x over previous
"""Trainium2 Bass kernel for nn_ContrastLoss_Disentangle.

Contract: kernel(**inputs) takes the FULL (unsharded) inputs and returns
(loss_label, loss_norm, loss_triple) as float32 scalars.

Pipeline (8 NeuronCores, data-parallel over pose/nlp rows):
  host:    norms (pose+nlp), normalization, X = nlpF * categories / norm
           (bf16, cd-partition interleaved layout), poseFT fp8 (x16) for the
           gram matrix, poseFT bf16 interleaved for the dot products
  device1: per-core: scores for the core's 1024 nlp rows (DVE bf16 mult +
           PE ones-matmul partition reduction) and a [512, 1024] block of
           the pose gram matrix (PE, fp8 DoubleRow), emitted as f16
  host:    BCE, pm assembly + stable argsort rank-select (furthest), gather
           of the hard-positive X columns
  device2: per-core: cur dots of the gathered X columns (same structure)
  host:    triplet loss assembly
"""

import os
import numpy as np
import ml_dtypes

import concourse.bass as bass
import concourse.tile as tile
from concourse import bacc, mybir
from concourse.bass2jax import install_neuronx_cc_hook, partition_id_tensor, _bass_exec_p

C, NP, K, D = 8, 2048, 4, 256
NN = NP * K          # 8192
NCORES = 8
NPL = NP // NCORES   # 256 poses per core
NNL = NN * 1 // NCORES  # 1024 nlp rows per core
CD = C * D           # 2048 contraction size
KC = CD // 128       # 16 cd chunks of 128 partitions

# pm block grid: 4 row-blocks x 2 col-blocks
PM_MI, PM_NJ = 4, 2
PM_M = NP // PM_MI   # 512 rows per core block
PM_N = NP // PM_NJ   # 1024 cols per core block
KT2 = CD // 256      # 8 DoubleRow contraction chunks

PM_SCALE = 16.0      # fp8 pre-scale (argsort is scale-invariant)

BF16 = ml_dtypes.bfloat16
FP8 = ml_dtypes.float8_e4m3

_runners = {}


def _build_kernel(with_pm: bool):
    """Per-core program. Inputs (per core):
      xtp [128, KC, K+1, NPL] bf16  cd-interleaved: slots 0..3 = X columns
          ((k, pose)-major), slot 4 = poseFT columns of this core
      pml [CD, PM_M] fp8, pmr [CD, PM_N] fp8  (only when with_pm)
    Outputs:
      sc  [1, K*NPL] f32   scores, col = k*NPL + p_local
      pmblk [PM_M, PM_N] f16  (only when with_pm)
    """
    f8 = mybir.dt.float8e4
    bf = mybir.dt.bfloat16
    f16 = mybir.dt.float16
    f32 = mybir.dt.float32
    nc = bacc.Bacc("TRN2", target_bir_lowering=False, debug=False,
                   num_devices=NCORES)
    xtp = nc.dram_tensor("xtp", [128, KC, K + 1, NPL], bf,
                         kind="ExternalInput").ap()
    if with_pm:
        pml = nc.dram_tensor("pml", [CD, PM_M], f8, kind="ExternalInput").ap()
        pmr = nc.dram_tensor("pmr", [CD, PM_N], f8, kind="ExternalInput").ap()
        pmblk = nc.dram_tensor("pmblk", [PM_M, PM_N], f16,
                               kind="ExternalOutput").ap()
    sc = nc.dram_tensor("sc", [1, K * NPL], f32, kind="ExternalOutput").ap()

    DR = mybir.MatmulPerfMode.DoubleRow
    with tile.TileContext(nc) as tc:
        with tc.tile_pool(name="cst", bufs=1) as cst, \
             tc.tile_pool(name="zp", bufs=3) as zp, \
             tc.tile_pool(name="ev", bufs=2) as ev, \
             tc.tile_pool(name="ps", bufs=1, space="PSUM") as ps, \
             tc.tile_pool(name="ps_s", bufs=1, space="PSUM") as ps_s:

            ones = cst.tile([128, 1], bf, tag="ones")
            nc.gpsimd.memset(ones[:], 1.0)

            acc_s = [ps_s.tile([1, 512], f32, tag=f"accs{h}", name=f"accs{h}")
                     for h in range(2)]

            # ---------- scores sub-program (one cd chunk) -----------------
            XCH = 4  # kc chunks per xtp DMA
            xt_t = cst.tile([128, KC, K + 1, NPL], bf, tag="xt")

            def x_load(ci):
                nc.sync.dma_start(xt_t[:, XCH * ci:XCH * (ci + 1)],
                                  xtp[:, XCH * ci:XCH * (ci + 1)])

            def s_iter(kc):
                z = zp.tile([128, K, NPL], bf, tag="z", name=f"z{kc}")
                full = xt_t[:, kc, K]
                rep = bass.AP(tensor=full.tensor, offset=full.offset,
                              ap=[list(full.ap[0]), [0, K], list(full.ap[1])])
                nc.vector.tensor_tensor(z[:], xt_t[:, kc, 0:K], rep,
                                        op=mybir.AluOpType.mult)
                zf = z[:].rearrange("p k n -> p (k n)")
                for h in range(2):
                    nc.tensor.matmul(acc_s[h][:], ones[:],
                                     zf[:, 512 * h:512 * (h + 1)],
                                     start=(kc == 0), stop=(kc == KC - 1),
                                     skip_group_check=True)

            def s_flush():
                o = ev.tile([1, K * NPL], f32, tag="so")
                for h in range(2):
                    nc.scalar.copy(o[:, 512 * h:512 * (h + 1)], acc_s[h][:])
                nc.sync.dma_start(sc[:], o[:])

            # ---------- pm sub-program ------------------------------------
            if with_pm:
                lt = cst.tile([128, KT2, 2, PM_M], f8, tag="lt")
                rt = cst.tile([128, KT2, 2, PM_N], f8, tag="rt")
                acc = ps.tile([128, 4, 512], f32, tag="acc", name="acc")

                def lt_load(half):
                    nc.sync.dma_start(
                        lt[:, 4 * half:4 * (half + 1)],
                        pml[1024 * half:1024 * (half + 1), :]
                        .rearrange("(kc t p) m -> p kc t m", p=128, t=2))

                def rt_load(n):
                    nc.sync.dma_start(
                        rt[:, :, :, 512 * n:512 * (n + 1)],
                        pmr.rearrange("(kc t p) q -> p kc t q", p=128, t=2)
                        [:, :, :, 512 * n:512 * (n + 1)])

                def pm_chunk(s):
                    n, kc = s // KT2, s % KT2
                    for m in range(PM_MI):
                        nc.tensor.matmul(acc[:, m][:],
                                         lt[:, kc, :, 128 * m:128 * (m + 1)],
                                         rt[:, kc, :, 512 * n:512 * (n + 1)],
                                         start=(kc == 0),
                                         stop=(kc == KT2 - 1),
                                         perf_mode=DR, skip_group_check=True)
                    if kc == KT2 - 1:
                        o = ev.tile([128, 4, 512], f16, tag="ev",
                                    name=f"ev{n}")
                        for m in range(PM_MI):
                            nc.scalar.copy(o[:, m][:], acc[:, m][:])
                        nc.sync.dma_start(
                            pmblk.rearrange("(m p) q -> p m q", p=128)
                            [:, :, 512 * n:512 * (n + 1)], o[:])

            # ---------- emission order ------------------------------------
            if with_pm:
                x_load(0)
                s_iter(0)
                lt_load(0)
                lt_load(1)
                rt_load(0)
                s_iter(1)
                x_load(1)
                for s in range(KT2):          # pm n=0
                    pm_chunk(s)
                    if s == 0:
                        rt_load(1)
                    if s == 1:
                        x_load(2)
                    if 2 + s < KC:
                        s_iter(2 + s)
                x_load(3)
                for s in range(KT2, 2 * KT2):  # pm n=1
                    pm_chunk(s)
                    if 2 + s < KC:
                        s_iter(2 + s)
                for kc in range(2 + 2 * KT2, KC):
                    s_iter(kc)
                s_flush()
            else:
                for ci in range(KC // XCH):
                    x_load(ci)
                    for kc in range(XCH * ci, XCH * (ci + 1)):
                        s_iter(kc)
                s_flush()

    nc.finalize()
    return nc


def _make_runner(nc):
    """Reusable jitted SPMD runner (replicates bass2jax.run_bass_via_pjrt but
    caches the compiled executable across calls)."""
    import jax
    from jax.sharding import Mesh, PartitionSpec
    from jax.experimental.shard_map import shard_map

    install_neuronx_cc_hook()
    partition_name = nc.partition_id_tensor.name if nc.partition_id_tensor else None
    in_names, out_names, out_avals = [], [], []
    for alloc in nc.m.functions[0].allocations:
        if not isinstance(alloc, mybir.MemoryLocationSet):
            continue
        name = alloc.memorylocations[0].name
        if alloc.kind == "ExternalInput":
            if name != partition_name:
                in_names.append(name)
        elif alloc.kind == "ExternalOutput":
            out_names.append(name)
            out_avals.append(jax.core.ShapedArray(
                tuple(alloc.tensor_shape), mybir.dt.np(alloc.dtype)))
    n_params = len(in_names)
    all_in = in_names + out_names + ([partition_name] if partition_name else [])

    def _body(*args):
        operands = list(args)
        if partition_name is not None:
            operands.append(partition_id_tensor())
        outs = _bass_exec_p.bind(
            *operands, out_avals=tuple(out_avals), in_names=tuple(all_in),
            out_names=tuple(out_names), lowering_input_output_aliases=(),
            sim_require_finite=False, sim_require_nnan=False, nc=nc)
        return tuple(outs)

    devices = jax.devices()[:NCORES]
    mesh = Mesh(np.asarray(devices), ("core",))
    donate = tuple(range(n_params, n_params + len(out_names)))
    sharded = jax.jit(
        shard_map(_body, mesh=mesh,
                  in_specs=(PartitionSpec("core"),) * (n_params + len(out_names)),
                  out_specs=(PartitionSpec("core"),) * len(out_names),
                  check_rep=False),
        donate_argnums=donate, keep_unused=True)

    def run(in_maps):
        concat_in = [np.concatenate([np.asarray(m[name]) for m in in_maps], axis=0)
                     for name in in_names]
        zeros = [np.zeros((NCORES * a.shape[0], *a.shape[1:]), a.dtype)
                 for a in out_avals]
        out_arrs = sharded(*concat_in, *zeros)
        return [
            {name: np.asarray(out_arrs[i]).reshape(NCORES, *out_avals[i].shape)[c]
             for i, name in enumerate(out_names)}
            for c in range(NCORES)
        ]

    return run


def _get_runner(key):
    if key not in _runners:
        _runners[key] = _make_runner(_build_kernel(with_pm=(key == "k1")))
    return _runners[key]


def _interleave_cols(A):
    """[CD, cols] -> [128, KC, cols]: partition p holds cd rows kc*128+p."""
    return np.ascontiguousarray(
        A.reshape(KC, 128, A.shape[1]).transpose(1, 0, 2))


def _x_dev(Xt16, cols):
    """Columns `cols` (global nlp ids, (k, pose)-major order per core) of the
    bf16 [CD, NN] X^T matrix -> [128, KC, K, NPL] device layout."""
    return _interleave_cols(Xt16[:, cols]).reshape(128, KC, K, NPL)


def _scores_from_dev(res, name="sc"):
    """Per-core [1, K*NPL] (col = k*NPL + p_local) -> [NN] global scores."""
    out = np.empty((NCORES, NPL, K), np.float32)
    for c in range(NCORES):
        out[c] = res[c][name].reshape(K, NPL).T
    return out.reshape(NN)


def _kernel_host_fallback(inputs):
    """Pure-numpy reference replication, used only if the index tensors do
    not have the canonical arange structure the device layout relies on."""
    nlp = np.asarray(inputs["nlp_features"], np.float32)
    pose = np.asarray(inputs["pose_features"], np.float32)
    nlab = np.asarray(inputs["nlp_label"]).astype(np.int64)
    n2p = np.asarray(inputs["nlpid2poseid"]).astype(np.int64)
    p2n = np.asarray(inputs["pose2nlpid"]).astype(np.int64)
    cat = np.asarray(inputs["categories"], np.float32)
    ri = np.asarray(inputs["rand_index"]).astype(np.int64)
    Np, Nn = pose.shape[1], nlp.shape[1]
    norm_p = np.sqrt(np.einsum("cpd,cpd->cp", pose, pose, dtype=np.float32))
    norm_n = np.sqrt(np.einsum("cnd,cnd->cn", nlp, nlp, dtype=np.float32))
    poseF = pose / norm_p[:, :, None]
    nlpF = nlp / norm_n[:, :, None]
    loss_norm = np.float32(np.float32(norm_p.mean()) + np.float32(norm_n.mean()))
    dots = np.einsum("cnd,cnd->cn", nlpF, poseF[:, n2p]).astype(np.float32)
    scores = np.einsum("cn,nc->n", dots, cat).astype(np.float32)
    p = (1.0 / (1.0 + np.exp(-scores))).astype(np.float32)
    lblf = nlab.astype(np.float32)
    loss_label = np.float32(
        np.mean(-(np.log(p) * lblf + np.log(1.0 - p) * (1.0 - lblf))))
    pf = np.ascontiguousarray(poseF.transpose(0, 2, 1).reshape(-1, Np))
    pm = (pf.T @ pf).astype(np.float32)
    ar = np.arange(Np)
    pm[ar, ar] = 1.0
    order = np.argsort(pm, axis=1, kind="stable")
    furthest = order[ar, ri]
    sg = scores[p2n]
    lg = nlab[p2n]
    maxp = np.maximum(np.max(np.where(lg == 0, sg, -np.inf), axis=1), -1.0)
    minp = np.minimum(np.min(np.where(lg == 1, sg, np.inf), axis=1), 1.0)
    nids = p2n[furthest]
    cd = np.einsum("cpkd,cpd->cpk", nlpF[:, nids], poseF)
    cur = np.einsum("cpk,pkc->pk", cd, cat[nids]).astype(np.float32)
    lcur = nlab[nids]
    maxcur = np.max(np.where(lcur == 1, cur, -np.inf), axis=1)
    maxp = np.maximum(maxp, maxcur)
    found = ~((maxp == -1.0) | (minp == 1.0))
    lt = np.where(found, maxp - minp + 2.0, 0.0).astype(np.float32)
    nf = int(np.sum(~found))
    loss_triple = (np.float32(0.0) if nf == Nn else
                   np.float32(lt.sum(dtype=np.float32) / np.float32(Nn - nf)))
    return (np.float32(loss_label), loss_norm, loss_triple)


def kernel(**inputs):
    nlp = np.ascontiguousarray(inputs["nlp_features"], np.float32)      # [C, NN, D]
    pose = np.ascontiguousarray(inputs["pose_features"], np.float32)    # [C, NP, D]
    nlab = np.asarray(inputs["nlp_label"]).astype(np.int64)
    cat = np.ascontiguousarray(inputs["categories"], np.float32)        # [NN, C]
    ri = np.asarray(inputs["rand_index"]).astype(np.int64)

    n2p = np.asarray(inputs["nlpid2poseid"]).astype(np.int64)
    p2n = np.asarray(inputs["pose2nlpid"]).astype(np.int64)
    if (not np.array_equal(n2p, np.arange(NN) // K)
            or not np.array_equal(p2n, np.arange(NN).reshape(NP, K))):
        return _kernel_host_fallback(inputs)

    # ---- host: norms, X^T (bf16), poseFT (bf16 + fp8) --------------------
    norm_p = np.sqrt(np.einsum("cpd,cpd->cp", pose, pose, dtype=np.float32,
                               optimize=True)).astype(np.float32)       # [C, NP]
    norm_n = np.sqrt(np.einsum("cnd,cnd->cn", nlp, nlp, dtype=np.float32,
                               optimize=True)).astype(np.float32)       # [C, NN]
    loss_norm = np.float32(np.float32(norm_p.mean()) + np.float32(norm_n.mean()))

    poseF = pose / norm_p[:, :, None]
    poseFT = np.ascontiguousarray(
        poseF.transpose(0, 2, 1).reshape(CD, NP))                       # [CD, NP]
    P8 = (poseFT * PM_SCALE).astype(FP8)                                # [CD, NP]

    scale = (cat.T / norm_n).astype(np.float32)                         # [C, NN]
    Xt16 = np.ascontiguousarray(
        (nlp * scale[:, :, None]).transpose(0, 2, 1).reshape(CD, NN)
    ).astype(BF16)                                                      # [CD, NN]

    # (k, pose)-major column order within each core's 1024 nlp columns
    base_cols = (np.arange(NPL)[None, :] * K + np.arange(K)[:, None]).reshape(-1)

    # ---- device kernel 1: scores + pm blocks -----------------------------
    run1 = _get_runner("k1")
    po_dev = [np.ascontiguousarray(_interleave_cols(
        np.ascontiguousarray(poseFT[:, c * NPL:(c + 1) * NPL]).astype(BF16))
        [:, :, None, :]) for c in range(NCORES)]
    in_maps = []
    for c in range(NCORES):
        i, j = c // PM_NJ, c % PM_NJ
        in_maps.append({
            "xtp": np.concatenate(
                [_x_dev(Xt16, c * NNL + base_cols), po_dev[c]], axis=2),
            "pml": np.ascontiguousarray(P8[:, i * PM_M:(i + 1) * PM_M]),
            "pmr": np.ascontiguousarray(P8[:, j * PM_N:(j + 1) * PM_N]),
        })
    res1 = run1(in_maps)

    scores = _scores_from_dev(res1)                                     # [NN]
    pm = np.empty((NP, NP), np.float32)
    for c in range(NCORES):
        i, j = c // PM_NJ, c % PM_NJ
        pm[i * PM_M:(i + 1) * PM_M, j * PM_N:(j + 1) * PM_N] = \
            res1[c]["pmblk"].astype(np.float32)

    # ---- host: BCE -------------------------------------------------------
    p = (1.0 / (1.0 + np.exp(-scores))).astype(np.float32)
    lblf = nlab.astype(np.float32)
    loss_label = np.float32(
        np.mean(-(np.log(p) * lblf + np.log(1.0 - p) * (1.0 - lblf))))

    # ---- host: furthest selection (pm is PM_SCALE^2 * gram) --------------
    ar = np.arange(NP)
    pm[ar, ar] = PM_SCALE * PM_SCALE
    order = np.argsort(pm, axis=1, kind="stable")
    furthest = order[ar, ri]                                            # [NP]

    sg = scores.reshape(NP, K)
    lg = nlab.reshape(NP, K)
    maxp = np.maximum(np.max(np.where(lg == 0, sg, -np.inf), axis=1), -1.0)
    minp = np.minimum(np.min(np.where(lg == 1, sg, np.inf), axis=1), 1.0)

    # ---- device kernel 2: dots of gathered hard-positive columns ---------
    run2 = _get_runner("k2")
    in_maps2 = []
    for c in range(NCORES):
        # col j = k*NPL + p_local -> global nlp id 4*furthest[p] + k
        fth = furthest[c * NPL:(c + 1) * NPL]
        cols = (np.arange(K)[:, None] + fth[None, :] * K).reshape(-1)
        in_maps2.append({
            "xtp": np.concatenate([_x_dev(Xt16, cols), po_dev[c]], axis=2),
        })
    res2 = run2(in_maps2)
    cur = _scores_from_dev(res2).reshape(NP, K)

    nids = (furthest[:, None] * K + np.arange(K)).reshape(-1)           # [NN]
    lcur = nlab[nids].reshape(NP, K)
    maxcur = np.max(np.where(lcur == 1, cur, -np.inf), axis=1)
    maxp = np.maximum(maxp, maxcur)
    found = ~((maxp == -1.0) | (minp == 1.0))
    lt = np.where(found, maxp - minp + 2.0, 0.0).astype(np.float32)
    not_find = int(np.sum(~found))
    if not_find == NN:
        loss_triple = np.float32(0.0)
    else:
        loss_triple = np.float32(lt.sum(dtype=np.float32) / np.float32(NN - not_find))

    return (np.float32(loss_label), np.float32(loss_norm), np.float32(loss_triple))


# revision 12
# speedup vs baseline: 2.2713x; 1.1212x over previous
"""Trainium2 Bass kernel for nn_ContrastLoss_Disentangle.

Contract: kernel(**inputs) takes the FULL (unsharded) inputs and returns
(loss_label, loss_norm, loss_triple) as float32 scalars.

Pipeline (8 NeuronCores, data-parallel over pose/nlp rows):
  host:    norms (pose+nlp), normalization, X = nlpF * categories / norm
           (bf16, cd-partition interleaved layout), poseFT fp8 (x16) for the
           gram matrix, poseFT bf16 interleaved for the dot products
  device1: per-core: scores for the core's 1024 nlp rows (DVE bf16 mult +
           PE ones-matmul partition reduction) and a [512, 1024] block of
           the pose gram matrix (PE, fp8 DoubleRow), emitted as f16
  host:    BCE, pm assembly + stable argsort rank-select (furthest), gather
           of the hard-positive X columns
  device2: per-core: cur dots of the gathered X columns (same structure)
  host:    triplet loss assembly
"""

import os
import numpy as np
import ml_dtypes

import concourse.bass as bass
import concourse.tile as tile
from concourse import bacc, mybir
from concourse.bass2jax import install_neuronx_cc_hook, partition_id_tensor, _bass_exec_p

C, NP, K, D = 8, 2048, 4, 256
NN = NP * K          # 8192
NCORES = 8
NPL = NP // NCORES   # 256 poses per core
NNL = NN * 1 // NCORES  # 1024 nlp rows per core
CD = C * D           # 2048 contraction size
KC = CD // 128       # 16 cd chunks of 128 partitions

# pm block grid: 4 row-blocks x 2 col-blocks
PM_MI, PM_NJ = 4, 2
PM_M = NP // PM_MI   # 512 rows per core block
PM_N = NP // PM_NJ   # 1024 cols per core block
KT2 = CD // 256      # 8 DoubleRow contraction chunks

PM_SCALE = 16.0      # fp8 pre-scale for pose columns (argsort is scale-invariant)
X_SCALE = 8.0        # fp8 pre-scale for X columns (scores come back x128)

BF16 = ml_dtypes.bfloat16
FP8 = ml_dtypes.float8_e4m3

_runners = {}


NM = 8  # m-chunks of 128 score columns per core


def _build_kernel(with_pm: bool):
    """Per-core program. Inputs (per core):
      x8  [128, NM, KT2, 2, 128] fp8  X columns (x8 scale), m-packed:
          x8[p, m, kc2, t, jj] = X8[kc2*256+t*128+p, col(m*128+jj)]
          with col() the core's (k, pose)-major order
      po8 [128, KT2, 2, NPL] fp8  pose columns (x16 scale), same packing
      msk [128, 2, 256] bf16  diagonal masks: msk[jj, par, q] = (q==jj+128*par)
      pml [CD, PM_M] fp8, pmr [CD, PM_N] fp8  (only when with_pm)
    Outputs:
      sc  [128, NM] f32   128*score[m*128+jj] at [jj, m]
      pmblk [PM_M, PM_N] f16  (only when with_pm)
    """
    f8 = mybir.dt.float8e4
    bf = mybir.dt.bfloat16
    f16 = mybir.dt.float16
    f32 = mybir.dt.float32
    nc = bacc.Bacc("TRN2", target_bir_lowering=False, debug=False,
                   num_devices=NCORES)
    x8 = nc.dram_tensor("x8", [128, NM, KT2, 2, 128], f8,
                        kind="ExternalInput").ap()
    po8 = nc.dram_tensor("po8", [128, KT2, 2, NPL], f8,
                         kind="ExternalInput").ap()
    msk = nc.dram_tensor("msk", [128, 2, 256], bf, kind="ExternalInput").ap()
    if with_pm:
        pml = nc.dram_tensor("pml", [CD, PM_M], f8, kind="ExternalInput").ap()
        pmr = nc.dram_tensor("pmr", [CD, PM_N], f8, kind="ExternalInput").ap()
        pmblk = nc.dram_tensor("pmblk", [PM_M, PM_N], f16,
                               kind="ExternalOutput").ap()
    sc = nc.dram_tensor("sc", [128, NM], f32, kind="ExternalOutput").ap()

    DR = mybir.MatmulPerfMode.DoubleRow
    with tile.TileContext(nc) as tc:
        with tc.tile_pool(name="cst", bufs=1) as cst, \
             tc.tile_pool(name="zp", bufs=3) as zp, \
             tc.tile_pool(name="ev", bufs=2) as ev, \
             tc.tile_pool(name="ps", bufs=1, space="PSUM") as ps, \
             tc.tile_pool(name="ps_s", bufs=2, space="PSUM") as ps_s:

            msk_t = cst.tile([128, 2, 256], bf, tag="msk")
            nc.sync.dma_start(msk_t[:], msk)
            po8_t = cst.tile([128, KT2, 2, NPL], f8, tag="po8")
            nc.sync.dma_start(po8_t[:], po8)
            x8_t = cst.tile([128, NM, KT2, 2, 128], f8, tag="x8")
            sct = cst.tile([128, NM], f32, tag="sct")

            XCH = 2  # m-chunks per x8 DMA

            def x_load(ci):
                nc.sync.dma_start(x8_t[:, XCH * ci:XCH * (ci + 1)],
                                  x8[:, XCH * ci:XCH * (ci + 1)])

            def scores_m(m):
                psc = ps_s.tile([128, 256], f32, tag="psc", name=f"psc{m}")
                for kc2 in range(KT2):
                    nc.tensor.matmul(psc[:], x8_t[:, m, kc2], po8_t[:, kc2],
                                     start=(kc2 == 0), stop=(kc2 == KT2 - 1),
                                     perf_mode=DR, skip_group_check=True)
                zm = zp.tile([128, 256], bf, tag="zm", name=f"zm{m}")
                nc.vector.tensor_tensor(zm[:], psc[:], msk_t[:, m & 1],
                                        op=mybir.AluOpType.mult)
                nc.vector.tensor_reduce(sct[:, m:m + 1], zm[:],
                                        axis=mybir.AxisListType.X,
                                        op=mybir.AluOpType.add)

            def s_flush():
                nc.sync.dma_start(sc, sct[:])

            # ---------- pm sub-program ------------------------------------
            if with_pm:
                lt = cst.tile([128, KT2, 2, PM_M], f8, tag="lt")
                rt = cst.tile([128, KT2, 2, PM_N], f8, tag="rt")
                acc = ps.tile([128, 4, 512], f32, tag="acc", name="acc")

                def lt_load(half):
                    nc.sync.dma_start(
                        lt[:, 4 * half:4 * (half + 1)],
                        pml[1024 * half:1024 * (half + 1), :]
                        .rearrange("(kc t p) m -> p kc t m", p=128, t=2))

                def rt_load(n):
                    nc.sync.dma_start(
                        rt[:, :, :, 512 * n:512 * (n + 1)],
                        pmr.rearrange("(kc t p) q -> p kc t q", p=128, t=2)
                        [:, :, :, 512 * n:512 * (n + 1)])

                def pm_chunk(s):
                    n, kc = s // KT2, s % KT2
                    for m in range(PM_MI):
                        nc.tensor.matmul(acc[:, m][:],
                                         lt[:, kc, :, 128 * m:128 * (m + 1)],
                                         rt[:, kc, :, 512 * n:512 * (n + 1)],
                                         start=(kc == 0),
                                         stop=(kc == KT2 - 1),
                                         perf_mode=DR, skip_group_check=True)
                    if kc == KT2 - 1:
                        o = ev.tile([128, 4, 512], f16, tag="ev",
                                    name=f"ev{n}")
                        for m in range(PM_MI):
                            nc.scalar.copy(o[:, m][:], acc[:, m][:])
                        nc.sync.dma_start(
                            pmblk.rearrange("(m p) q -> p m q", p=128)
                            [:, :, 512 * n:512 * (n + 1)], o[:])

            # ---------- emission order ------------------------------------
            if with_pm:
                lt_load(0)
                lt_load(1)
                rt_load(0)
                x_load(0)
                for s in range(KT2):          # pm n=0
                    pm_chunk(s)
                    if s == 0:
                        rt_load(1)
                    if s == 1:
                        x_load(1)
                    if s == 3:
                        scores_m(0)
                    if s == 5:
                        scores_m(1)
                x_load(2)
                scores_m(2)
                for s in range(KT2, 2 * KT2):  # pm n=1
                    pm_chunk(s)
                    if s == KT2 + 1:
                        scores_m(3)
                    if s == KT2 + 3:
                        x_load(3)
                        scores_m(4)
                    if s == KT2 + 5:
                        scores_m(5)
                scores_m(6)
                scores_m(7)
                s_flush()
            else:
                for ci in range(NM // XCH):
                    x_load(ci)
                    for m in range(XCH * ci, XCH * (ci + 1)):
                        scores_m(m)
                s_flush()

    nc.finalize()
    return nc


def _make_runner(nc):
    """Reusable jitted SPMD runner (replicates bass2jax.run_bass_via_pjrt but
    caches the compiled executable across calls)."""
    import jax
    from jax.sharding import Mesh, PartitionSpec
    from jax.experimental.shard_map import shard_map

    install_neuronx_cc_hook()
    partition_name = nc.partition_id_tensor.name if nc.partition_id_tensor else None
    in_names, out_names, out_avals = [], [], []
    for alloc in nc.m.functions[0].allocations:
        if not isinstance(alloc, mybir.MemoryLocationSet):
            continue
        name = alloc.memorylocations[0].name
        if alloc.kind == "ExternalInput":
            if name != partition_name:
                in_names.append(name)
        elif alloc.kind == "ExternalOutput":
            out_names.append(name)
            out_avals.append(jax.core.ShapedArray(
                tuple(alloc.tensor_shape), mybir.dt.np(alloc.dtype)))
    n_params = len(in_names)
    all_in = in_names + out_names + ([partition_name] if partition_name else [])

    def _body(*args):
        operands = list(args)
        if partition_name is not None:
            operands.append(partition_id_tensor())
        outs = _bass_exec_p.bind(
            *operands, out_avals=tuple(out_avals), in_names=tuple(all_in),
            out_names=tuple(out_names), lowering_input_output_aliases=(),
            sim_require_finite=False, sim_require_nnan=False, nc=nc)
        return tuple(outs)

    devices = jax.devices()[:NCORES]
    mesh = Mesh(np.asarray(devices), ("core",))
    donate = tuple(range(n_params, n_params + len(out_names)))
    sharded = jax.jit(
        shard_map(_body, mesh=mesh,
                  in_specs=(PartitionSpec("core"),) * (n_params + len(out_names)),
                  out_specs=(PartitionSpec("core"),) * len(out_names),
                  check_rep=False),
        donate_argnums=donate, keep_unused=True)

    def run(in_maps):
        concat_in = [np.concatenate([np.asarray(m[name]) for m in in_maps], axis=0)
                     for name in in_names]
        zeros = [np.zeros((NCORES * a.shape[0], *a.shape[1:]), a.dtype)
                 for a in out_avals]
        out_arrs = sharded(*concat_in, *zeros)
        return [
            {name: np.asarray(out_arrs[i]).reshape(NCORES, *out_avals[i].shape)[c]
             for i, name in enumerate(out_names)}
            for c in range(NCORES)
        ]

    return run


def _get_runner(key):
    if key not in _runners:
        _runners[key] = _make_runner(_build_kernel(with_pm=(key == "k1")))
    return _runners[key]


def _x_dev(X8, cols):
    """Columns `cols` (global nlp ids, (k, pose)-major order per core) of the
    fp8 [CD, NN] X^T matrix -> [128, NM, KT2, 2, 128] device layout."""
    A = X8[:, cols]                                # [CD, 1024]
    return np.ascontiguousarray(
        A.reshape(KT2, 2, 128, NM, 128).transpose(2, 3, 0, 1, 4))


def _po_dev(P8c):
    """[CD, NPL] fp8 pose columns -> [128, KT2, 2, NPL] device layout."""
    return np.ascontiguousarray(
        P8c.reshape(KT2, 2, 128, NPL).transpose(2, 0, 1, 3))


def _scores_from_dev(res, name="sc"):
    """Per-core [128, NM] (row jj, col m -> j = m*128+jj, scaled x128, with
    col j = k*NPL + p_local) -> [NN] global scores."""
    out = np.empty((NCORES, NPL, K), np.float32)
    for c in range(NCORES):
        sc = np.asarray(res[c][name], np.float32)
        out[c] = (sc.T.reshape(K * NPL) / 128.0).reshape(K, NPL).T
    return out.reshape(NN)


def _kernel_host_fallback(inputs):
    """Pure-numpy reference replication, used only if the index tensors do
    not have the canonical arange structure the device layout relies on."""
    nlp = np.asarray(inputs["nlp_features"], np.float32)
    pose = np.asarray(inputs["pose_features"], np.float32)
    nlab = np.asarray(inputs["nlp_label"]).astype(np.int64)
    n2p = np.asarray(inputs["nlpid2poseid"]).astype(np.int64)
    p2n = np.asarray(inputs["pose2nlpid"]).astype(np.int64)
    cat = np.asarray(inputs["categories"], np.float32)
    ri = np.asarray(inputs["rand_index"]).astype(np.int64)
    Np, Nn = pose.shape[1], nlp.shape[1]
    norm_p = np.sqrt(np.einsum("cpd,cpd->cp", pose, pose, dtype=np.float32))
    norm_n = np.sqrt(np.einsum("cnd,cnd->cn", nlp, nlp, dtype=np.float32))
    poseF = pose / norm_p[:, :, None]
    nlpF = nlp / norm_n[:, :, None]
    loss_norm = np.float32(np.float32(norm_p.mean()) + np.float32(norm_n.mean()))
    dots = np.einsum("cnd,cnd->cn", nlpF, poseF[:, n2p]).astype(np.float32)
    scores = np.einsum("cn,nc->n", dots, cat).astype(np.float32)
    p = (1.0 / (1.0 + np.exp(-scores))).astype(np.float32)
    lblf = nlab.astype(np.float32)
    loss_label = np.float32(
        np.mean(-(np.log(p) * lblf + np.log(1.0 - p) * (1.0 - lblf))))
    pf = np.ascontiguousarray(poseF.transpose(0, 2, 1).reshape(-1, Np))
    pm = (pf.T @ pf).astype(np.float32)
    ar = np.arange(Np)
    pm[ar, ar] = 1.0
    order = np.argsort(pm, axis=1, kind="stable")
    furthest = order[ar, ri]
    sg = scores[p2n]
    lg = nlab[p2n]
    maxp = np.maximum(np.max(np.where(lg == 0, sg, -np.inf), axis=1), -1.0)
    minp = np.minimum(np.min(np.where(lg == 1, sg, np.inf), axis=1), 1.0)
    nids = p2n[furthest]
    cd = np.einsum("cpkd,cpd->cpk", nlpF[:, nids], poseF)
    cur = np.einsum("cpk,pkc->pk", cd, cat[nids]).astype(np.float32)
    lcur = nlab[nids]
    maxcur = np.max(np.where(lcur == 1, cur, -np.inf), axis=1)
    maxp = np.maximum(maxp, maxcur)
    found = ~((maxp == -1.0) | (minp == 1.0))
    lt = np.where(found, maxp - minp + 2.0, 0.0).astype(np.float32)
    nf = int(np.sum(~found))
    loss_triple = (np.float32(0.0) if nf == Nn else
                   np.float32(lt.sum(dtype=np.float32) / np.float32(Nn - nf)))
    return (np.float32(loss_label), loss_norm, loss_triple)


def kernel(**inputs):
    nlp = np.ascontiguousarray(inputs["nlp_features"], np.float32)      # [C, NN, D]
    pose = np.ascontiguousarray(inputs["pose_features"], np.float32)    # [C, NP, D]
    nlab = np.asarray(inputs["nlp_label"]).astype(np.int64)
    cat = np.ascontiguousarray(inputs["categories"], np.float32)        # [NN, C]
    ri = np.asarray(inputs["rand_index"]).astype(np.int64)

    n2p = np.asarray(inputs["nlpid2poseid"]).astype(np.int64)
    p2n = np.asarray(inputs["pose2nlpid"]).astype(np.int64)
    if (not np.array_equal(n2p, np.arange(NN) // K)
            or not np.array_equal(p2n, np.arange(NN).reshape(NP, K))):
        return _kernel_host_fallback(inputs)

    # ---- host: norms, X^T (bf16), poseFT (bf16 + fp8) --------------------
    norm_p = np.sqrt(np.einsum("cpd,cpd->cp", pose, pose, dtype=np.float32,
                               optimize=True)).astype(np.float32)       # [C, NP]
    norm_n = np.sqrt(np.einsum("cnd,cnd->cn", nlp, nlp, dtype=np.float32,
                               optimize=True)).astype(np.float32)       # [C, NN]
    loss_norm = np.float32(np.float32(norm_p.mean()) + np.float32(norm_n.mean()))

    poseF = pose / norm_p[:, :, None]
    poseFT = np.ascontiguousarray(
        poseF.transpose(0, 2, 1).reshape(CD, NP))                       # [CD, NP]
    P8 = (poseFT * PM_SCALE).astype(FP8)                                # [CD, NP]

    scale = (cat.T / norm_n).astype(np.float32)                         # [C, NN]
    X8 = ((nlp * scale[:, :, None]).transpose(0, 2, 1).reshape(CD, NN)
          * X_SCALE).astype(FP8)                                        # [CD, NN]

    # (k, pose)-major column order within each core's 1024 nlp columns
    base_cols = (np.arange(NPL)[None, :] * K + np.arange(K)[:, None]).reshape(-1)

    mskv = np.zeros((128, 2, 256), BF16)
    jj = np.arange(128)
    mskv[jj, 0, jj] = 1
    mskv[jj, 1, jj + 128] = 1

    # ---- device kernel 1: scores + pm blocks -----------------------------
    run1 = _get_runner("k1")
    po_dev = [_po_dev(np.ascontiguousarray(P8[:, c * NPL:(c + 1) * NPL]))
              for c in range(NCORES)]
    in_maps = []
    for c in range(NCORES):
        i, j = c // PM_NJ, c % PM_NJ
        in_maps.append({
            "x8": _x_dev(X8, c * NNL + base_cols),
            "po8": po_dev[c],
            "msk": mskv,
            "pml": np.ascontiguousarray(P8[:, i * PM_M:(i + 1) * PM_M]),
            "pmr": np.ascontiguousarray(P8[:, j * PM_N:(j + 1) * PM_N]),
        })
    res1 = run1(in_maps)

    scores = _scores_from_dev(res1)                                     # [NN]
    pm = np.empty((NP, NP), np.float32)
    for c in range(NCORES):
        i, j = c // PM_NJ, c % PM_NJ
        pm[i * PM_M:(i + 1) * PM_M, j * PM_N:(j + 1) * PM_N] = \
            res1[c]["pmblk"].astype(np.float32)

    # ---- host: BCE -------------------------------------------------------
    p = (1.0 / (1.0 + np.exp(-scores))).astype(np.float32)
    lblf = nlab.astype(np.float32)
    loss_label = np.float32(
        np.mean(-(np.log(p) * lblf + np.log(1.0 - p) * (1.0 - lblf))))

    # ---- host: furthest selection (pm is PM_SCALE^2 * gram) --------------
    ar = np.arange(NP)
    pm[ar, ar] = PM_SCALE * PM_SCALE
    order = np.argsort(pm, axis=1, kind="stable")
    furthest = order[ar, ri]                                            # [NP]

    sg = scores.reshape(NP, K)
    lg = nlab.reshape(NP, K)
    maxp = np.maximum(np.max(np.where(lg == 0, sg, -np.inf), axis=1), -1.0)
    minp = np.minimum(np.min(np.where(lg == 1, sg, np.inf), axis=1), 1.0)

    # ---- device kernel 2: dots of gathered hard-positive columns ---------
    run2 = _get_runner("k2")
    in_maps2 = []
    for c in range(NCORES):
        # col j = k*NPL + p_local -> global nlp id 4*furthest[p] + k
        fth = furthest[c * NPL:(c + 1) * NPL]
        cols = (np.arange(K)[:, None] + fth[None, :] * K).reshape(-1)
        in_maps2.append({
            "x8": _x_dev(X8, cols),
            "po8": po_dev[c],
            "msk": mskv,
        })
    res2 = run2(in_maps2)
    cur = _scores_from_dev(res2).reshape(NP, K)

    nids = (furthest[:, None] * K + np.arange(K)).reshape(-1)           # [NN]
    lcur = nlab[nids].reshape(NP, K)
    maxcur = np.max(np.where(lcur == 1, cur, -np.inf), axis=1)
    maxp = np.maximum(maxp, maxcur)
    found = ~((maxp == -1.0) | (minp == 1.0))
    lt = np.where(found, maxp - minp + 2.0, 0.0).astype(np.float32)
    not_find = int(np.sum(~found))
    if not_find == NN:
        loss_triple = np.float32(0.0)
    else:
        loss_triple = np.float32(lt.sum(dtype=np.float32) / np.float32(NN - not_find))

    return (np.float32(loss_label), np.float32(loss_norm), np.float32(loss_triple))


# revision 16
# speedup vs baseline: 2.5158x; 1.1077x over previous
"""Trainium2 Bass kernel for nn_ContrastLoss_Disentangle.

Contract: kernel(**inputs) takes the FULL (unsharded) inputs and returns
(loss_label, loss_norm, loss_triple) as float32 scalars.

Pipeline (8 NeuronCores, data-parallel over pose/nlp rows):
  host:    norms (pose+nlp), normalization, X = nlpF * categories / norm
           (bf16, cd-partition interleaved layout), poseFT fp8 (x16) for the
           gram matrix, poseFT bf16 interleaved for the dot products
  device1: per-core: scores for the core's 1024 nlp rows (DVE bf16 mult +
           PE ones-matmul partition reduction) and a [512, 1024] block of
           the pose gram matrix (PE, fp8 DoubleRow), emitted as f16
  host:    BCE, pm assembly + stable argsort rank-select (furthest), gather
           of the hard-positive X columns
  device2: per-core: cur dots of the gathered X columns (same structure)
  host:    triplet loss assembly
"""

import os
import numpy as np
import ml_dtypes

import concourse.bass as bass
import concourse.tile as tile
from concourse import bacc, mybir
from concourse.bass2jax import install_neuronx_cc_hook, partition_id_tensor, _bass_exec_p

C, NP, K, D = 8, 2048, 4, 256
NN = NP * K          # 8192
NCORES = 8
NPL = NP // NCORES   # 256 poses per core
NNL = NN * 1 // NCORES  # 1024 nlp rows per core
CD = C * D           # 2048 contraction size
KC = CD // 128       # 16 cd chunks of 128 partitions

# pm block grid: 4 row-blocks x 2 col-blocks
PM_MI, PM_NJ = 4, 2
PM_M = NP // PM_MI   # 512 rows per core block
PM_N = NP // PM_NJ   # 1024 cols per core block
KT2 = CD // 256      # 8 DoubleRow contraction chunks

PM_SCALE = 16.0      # fp8 pre-scale for pose columns (argsort is scale-invariant)
X_SCALE = 8.0        # fp8 pre-scale for X columns (scores come back x128)

BF16 = ml_dtypes.bfloat16
FP8 = ml_dtypes.float8_e4m3

_runners = {}


NM = 8  # m-chunks of 128 score columns per core


def _build_kernel(with_pm: bool):
    """Per-core program. Inputs (per core):
      x8  [128, NM, KT2, 2, 128] fp8  X columns (x8 scale), m-packed:
          x8[p, m, kc2, t, jj] = X8[kc2*256+t*128+p, col(m*128+jj)]
          with col() the core's (k, pose)-major order
      po8 [128, KT2, 2, NPL] fp8  pose columns (x16 scale), same packing
      msk [128, 2, 256] bf16  diagonal masks: msk[jj, par, q] = (q==jj+128*par)
      pml [CD, PM_M] fp8, pmr [CD, PM_N] fp8  (only when with_pm)
    Outputs:
      sc  [128, NM] f32   128*score[m*128+jj] at [jj, m]
      pmblk [PM_M, PM_N] f16  (only when with_pm)
    """
    f8 = mybir.dt.float8e4
    f16 = mybir.dt.float16
    f32 = mybir.dt.float32
    nc = bacc.Bacc("TRN2", target_bir_lowering=False, debug=False,
                   num_devices=NCORES)
    x8 = nc.dram_tensor("x8", [128, NM, KT2, 2, 128], f8,
                        kind="ExternalInput").ap()
    # po8x: slots 0..KT2-1 = pose columns; slot KT2 = diagonal masks
    po8x = nc.dram_tensor("po8x", [128, KT2 + 1, 2, NPL], f8,
                          kind="ExternalInput").ap()
    if with_pm:
        pml = nc.dram_tensor("pml", [CD, PM_M], f8, kind="ExternalInput").ap()
        pmr = nc.dram_tensor("pmr", [CD, PM_N], f8, kind="ExternalInput").ap()
        pmblk = nc.dram_tensor("pmblk", [PM_M, PM_N], f16,
                               kind="ExternalOutput").ap()
    sc = nc.dram_tensor("sc", [128, NM], f16, kind="ExternalOutput").ap()

    DR = mybir.MatmulPerfMode.DoubleRow
    with tile.TileContext(nc) as tc:
        with tc.tile_pool(name="cst", bufs=1) as cst, \
             tc.tile_pool(name="zp", bufs=3) as zp, \
             tc.tile_pool(name="ev", bufs=2) as ev, \
             tc.tile_pool(name="ps", bufs=2, space="PSUM") as ps, \
             tc.tile_pool(name="ps_s", bufs=2, space="PSUM") as ps_s:

            po8_t = cst.tile([128, KT2 + 1, 2, NPL], f8, tag="po8")
            nc.sync.dma_start(po8_t[:], po8x)
            x8_t = cst.tile([128, NM, KT2, 2, 128], f8, tag="x8")
            sct = cst.tile([128, NM], f16, tag="sct")

            XCH = 2  # m-chunks per x8 DMA

            def x_load(ci):
                nc.sync.dma_start(x8_t[:, XCH * ci:XCH * (ci + 1)],
                                  x8[:, XCH * ci:XCH * (ci + 1)])

            def scores_m(m):
                psc = ps_s.tile([128, 256], f32, tag="psc", name=f"psc{m}")
                for kc2 in range(KT2):
                    nc.tensor.matmul(psc[:], x8_t[:, m, kc2],
                                     po8_t[:, kc2][:],
                                     start=(kc2 == 0), stop=(kc2 == KT2 - 1),
                                     perf_mode=DR, skip_group_check=True)
                zm = zp.tile([128, 256], f16, tag="zm", name=f"zm{m}")
                nc.vector.tensor_tensor(zm[:], psc[:], po8_t[:, KT2, m & 1],
                                        op=mybir.AluOpType.mult)
                with nc.allow_low_precision(reason="f16 scores ok"):
                    nc.vector.tensor_reduce(sct[:, m:m + 1], zm[:],
                                            axis=mybir.AxisListType.X,
                                            op=mybir.AluOpType.add)

            def s_flush():
                nc.sync.dma_start(sc, sct[:])

            # ---------- pm sub-program ------------------------------------
            if with_pm:
                lt = cst.tile([128, KT2, 2, PM_M], f8, tag="lt")
                rt = cst.tile([128, KT2, 2, PM_N], f8, tag="rt")

                def lt_load(half):
                    nc.sync.dma_start(
                        lt[:, 4 * half:4 * (half + 1)],
                        pml[1024 * half:1024 * (half + 1), :]
                        .rearrange("(kc t p) m -> p kc t m", p=128, t=2))

                def rt_load(n):
                    nc.sync.dma_start(
                        rt[:, :, :, 512 * n:512 * (n + 1)],
                        pmr.rearrange("(kc t p) q -> p kc t q", p=128, t=2)
                        [:, :, :, 512 * n:512 * (n + 1)])

                evs = {}

                def pm_mchain(n, m):
                    acc = ps.tile([128, 512], f32, tag="acc",
                                  name=f"acc{n}{m}")
                    for kc in range(KT2):
                        nc.tensor.matmul(acc[:],
                                         lt[:, kc, :, 128 * m:128 * (m + 1)],
                                         rt[:, kc, :, 512 * n:512 * (n + 1)],
                                         start=(kc == 0),
                                         stop=(kc == KT2 - 1),
                                         perf_mode=DR, skip_group_check=True)
                    if m == 0:
                        evs[n] = ev.tile([128, 4, 512], f16, tag="ev",
                                         name=f"ev{n}")
                    nc.scalar.copy(evs[n][:, m][:], acc[:])
                    if m == PM_MI - 1:
                        nc.sync.dma_start(
                            pmblk.rearrange("(m p) q -> p m q", p=128)
                            [:, :, 512 * n:512 * (n + 1)], evs[n][:])

            # ---------- emission order ------------------------------------
            if with_pm:
                po8_load_done = True
                lt_load(0)
                lt_load(1)
                rt_load(0)
                rt_load(1)
                x_load(0)
                pm_mchain(0, 0)
                pm_mchain(0, 1)
                x_load(1)
                scores_m(0)
                pm_mchain(0, 2)
                scores_m(1)
                pm_mchain(0, 3)
                x_load(2)
                scores_m(2)
                pm_mchain(1, 0)
                scores_m(3)
                pm_mchain(1, 1)
                x_load(3)
                scores_m(4)
                pm_mchain(1, 2)
                scores_m(5)
                pm_mchain(1, 3)
                scores_m(6)
                scores_m(7)
                s_flush()
            else:
                for ci in range(NM // XCH):
                    x_load(ci)
                    for m in range(XCH * ci, XCH * (ci + 1)):
                        scores_m(m)
                s_flush()

    nc.finalize()
    return nc


def _make_runner(nc):
    """Reusable jitted SPMD runner (replicates bass2jax.run_bass_via_pjrt but
    caches the compiled executable across calls)."""
    import jax
    from jax.sharding import Mesh, PartitionSpec
    from jax.experimental.shard_map import shard_map

    install_neuronx_cc_hook()
    partition_name = nc.partition_id_tensor.name if nc.partition_id_tensor else None
    in_names, out_names, out_avals = [], [], []
    for alloc in nc.m.functions[0].allocations:
        if not isinstance(alloc, mybir.MemoryLocationSet):
            continue
        name = alloc.memorylocations[0].name
        if alloc.kind == "ExternalInput":
            if name != partition_name:
                in_names.append(name)
        elif alloc.kind == "ExternalOutput":
            out_names.append(name)
            out_avals.append(jax.core.ShapedArray(
                tuple(alloc.tensor_shape), mybir.dt.np(alloc.dtype)))
    n_params = len(in_names)
    all_in = in_names + out_names + ([partition_name] if partition_name else [])

    def _body(*args):
        operands = list(args)
        if partition_name is not None:
            operands.append(partition_id_tensor())
        outs = _bass_exec_p.bind(
            *operands, out_avals=tuple(out_avals), in_names=tuple(all_in),
            out_names=tuple(out_names), lowering_input_output_aliases=(),
            sim_require_finite=False, sim_require_nnan=False, nc=nc)
        return tuple(outs)

    devices = jax.devices()[:NCORES]
    mesh = Mesh(np.asarray(devices), ("core",))
    donate = tuple(range(n_params, n_params + len(out_names)))
    sharded = jax.jit(
        shard_map(_body, mesh=mesh,
                  in_specs=(PartitionSpec("core"),) * (n_params + len(out_names)),
                  out_specs=(PartitionSpec("core"),) * len(out_names),
                  check_rep=False),
        donate_argnums=donate, keep_unused=True)

    def run(in_maps):
        concat_in = [np.concatenate([np.asarray(m[name]) for m in in_maps], axis=0)
                     for name in in_names]
        zeros = [np.zeros((NCORES * a.shape[0], *a.shape[1:]), a.dtype)
                 for a in out_avals]
        out_arrs = sharded(*concat_in, *zeros)
        return [
            {name: np.asarray(out_arrs[i]).reshape(NCORES, *out_avals[i].shape)[c]
             for i, name in enumerate(out_names)}
            for c in range(NCORES)
        ]

    return run


def _get_runner(key):
    if key not in _runners:
        _runners[key] = _make_runner(_build_kernel(with_pm=(key == "k1")))
    return _runners[key]


def _x_dev(X8, cols):
    """Columns `cols` (global nlp ids, (k, pose)-major order per core) of the
    fp8 [CD, NN] X^T matrix -> [128, NM, KT2, 2, 128] device layout."""
    A = X8[:, cols]                                # [CD, 1024]
    return np.ascontiguousarray(
        A.reshape(KT2, 2, 128, NM, 128).transpose(2, 3, 0, 1, 4))


def _po_dev(P8c, mskv):
    """[CD, NPL] fp8 pose columns -> [128, KT2+1, 2, NPL] device layout with
    the diagonal masks appended as slot KT2."""
    po = P8c.reshape(KT2, 2, 128, NPL).transpose(2, 0, 1, 3)
    return np.ascontiguousarray(
        np.concatenate([po, mskv[:, None, :, :]], axis=1))


def _scores_from_dev(res, name="sc"):
    """Per-core [128, NM] (row jj, col m -> j = m*128+jj, scaled x128, with
    col j = k*NPL + p_local) -> [NN] global scores."""
    out = np.empty((NCORES, NPL, K), np.float32)
    for c in range(NCORES):
        sc = np.asarray(res[c][name], np.float32)
        out[c] = (sc.T.reshape(K * NPL) / 128.0).reshape(K, NPL).T
    return out.reshape(NN)


def _kernel_host_fallback(inputs):
    """Pure-numpy reference replication, used only if the index tensors do
    not have the canonical arange structure the device layout relies on."""
    nlp = np.asarray(inputs["nlp_features"], np.float32)
    pose = np.asarray(inputs["pose_features"], np.float32)
    nlab = np.asarray(inputs["nlp_label"]).astype(np.int64)
    n2p = np.asarray(inputs["nlpid2poseid"]).astype(np.int64)
    p2n = np.asarray(inputs["pose2nlpid"]).astype(np.int64)
    cat = np.asarray(inputs["categories"], np.float32)
    ri = np.asarray(inputs["rand_index"]).astype(np.int64)
    Np, Nn = pose.shape[1], nlp.shape[1]
    norm_p = np.sqrt(np.einsum("cpd,cpd->cp", pose, pose, dtype=np.float32))
    norm_n = np.sqrt(np.einsum("cnd,cnd->cn", nlp, nlp, dtype=np.float32))
    poseF = pose / norm_p[:, :, None]
    nlpF = nlp / norm_n[:, :, None]
    loss_norm = np.float32(np.float32(norm_p.mean()) + np.float32(norm_n.mean()))
    dots = np.einsum("cnd,cnd->cn", nlpF, poseF[:, n2p]).astype(np.float32)
    scores = np.einsum("cn,nc->n", dots, cat).astype(np.float32)
    p = (1.0 / (1.0 + np.exp(-scores))).astype(np.float32)
    lblf = nlab.astype(np.float32)
    loss_label = np.float32(
        np.mean(-(np.log(p) * lblf + np.log(1.0 - p) * (1.0 - lblf))))
    pf = np.ascontiguousarray(poseF.transpose(0, 2, 1).reshape(-1, Np))
    pm = (pf.T @ pf).astype(np.float32)
    ar = np.arange(Np)
    pm[ar, ar] = 1.0
    order = np.argsort(pm, axis=1, kind="stable")
    furthest = order[ar, ri]
    sg = scores[p2n]
    lg = nlab[p2n]
    maxp = np.maximum(np.max(np.where(lg == 0, sg, -np.inf), axis=1), -1.0)
    minp = np.minimum(np.min(np.where(lg == 1, sg, np.inf), axis=1), 1.0)
    nids = p2n[furthest]
    cd = np.einsum("cpkd,cpd->cpk", nlpF[:, nids], poseF)
    cur = np.einsum("cpk,pkc->pk", cd, cat[nids]).astype(np.float32)
    lcur = nlab[nids]
    maxcur = np.max(np.where(lcur == 1, cur, -np.inf), axis=1)
    maxp = np.maximum(maxp, maxcur)
    found = ~((maxp == -1.0) | (minp == 1.0))
    lt = np.where(found, maxp - minp + 2.0, 0.0).astype(np.float32)
    nf = int(np.sum(~found))
    loss_triple = (np.float32(0.0) if nf == Nn else
                   np.float32(lt.sum(dtype=np.float32) / np.float32(Nn - nf)))
    return (np.float32(loss_label), loss_norm, loss_triple)


def kernel(**inputs):
    nlp = np.ascontiguousarray(inputs["nlp_features"], np.float32)      # [C, NN, D]
    pose = np.ascontiguousarray(inputs["pose_features"], np.float32)    # [C, NP, D]
    nlab = np.asarray(inputs["nlp_label"]).astype(np.int64)
    cat = np.ascontiguousarray(inputs["categories"], np.float32)        # [NN, C]
    ri = np.asarray(inputs["rand_index"]).astype(np.int64)

    n2p = np.asarray(inputs["nlpid2poseid"]).astype(np.int64)
    p2n = np.asarray(inputs["pose2nlpid"]).astype(np.int64)
    if (not np.array_equal(n2p, np.arange(NN) // K)
            or not np.array_equal(p2n, np.arange(NN).reshape(NP, K))):
        return _kernel_host_fallback(inputs)

    # ---- host: norms, X^T (bf16), poseFT (bf16 + fp8) --------------------
    norm_p = np.sqrt(np.einsum("cpd,cpd->cp", pose, pose, dtype=np.float32,
                               optimize=True)).astype(np.float32)       # [C, NP]
    norm_n = np.sqrt(np.einsum("cnd,cnd->cn", nlp, nlp, dtype=np.float32,
                               optimize=True)).astype(np.float32)       # [C, NN]
    loss_norm = np.float32(np.float32(norm_p.mean()) + np.float32(norm_n.mean()))

    poseF = pose / norm_p[:, :, None]
    poseFT = np.ascontiguousarray(
        poseF.transpose(0, 2, 1).reshape(CD, NP))                       # [CD, NP]
    P8 = (poseFT * PM_SCALE).astype(FP8)                                # [CD, NP]

    scale = (cat.T / norm_n).astype(np.float32)                         # [C, NN]
    X8 = ((nlp * scale[:, :, None]).transpose(0, 2, 1).reshape(CD, NN)
          * X_SCALE).astype(FP8)                                        # [CD, NN]

    # (k, pose)-major column order within each core's 1024 nlp columns
    base_cols = (np.arange(NPL)[None, :] * K + np.arange(K)[:, None]).reshape(-1)

    mskv = np.zeros((128, 2, 256), FP8)
    jj = np.arange(128)
    mskv[jj, 0, jj] = 1
    mskv[jj, 1, jj + 128] = 1

    # ---- device kernel 1: scores + pm blocks -----------------------------
    run1 = _get_runner("k1")
    po_dev = [_po_dev(np.ascontiguousarray(P8[:, c * NPL:(c + 1) * NPL]), mskv)
              for c in range(NCORES)]
    in_maps = []
    for c in range(NCORES):
        i, j = c // PM_NJ, c % PM_NJ
        in_maps.append({
            "x8": _x_dev(X8, c * NNL + base_cols),
            "po8x": po_dev[c],
            "pml": np.ascontiguousarray(P8[:, i * PM_M:(i + 1) * PM_M]),
            "pmr": np.ascontiguousarray(P8[:, j * PM_N:(j + 1) * PM_N]),
        })
    res1 = run1(in_maps)

    scores = _scores_from_dev(res1)                                     # [NN]
    pm = np.empty((NP, NP), np.float32)
    for c in range(NCORES):
        i, j = c // PM_NJ, c % PM_NJ
        pm[i * PM_M:(i + 1) * PM_M, j * PM_N:(j + 1) * PM_N] = \
            res1[c]["pmblk"].astype(np.float32)

    # ---- host: BCE -------------------------------------------------------
    p = (1.0 / (1.0 + np.exp(-scores))).astype(np.float32)
    lblf = nlab.astype(np.float32)
    loss_label = np.float32(
        np.mean(-(np.log(p) * lblf + np.log(1.0 - p) * (1.0 - lblf))))

    # ---- host: furthest selection (pm is PM_SCALE^2 * gram) --------------
    ar = np.arange(NP)
    pm[ar, ar] = PM_SCALE * PM_SCALE
    order = np.argsort(pm, axis=1, kind="stable")
    furthest = order[ar, ri]                                            # [NP]

    sg = scores.reshape(NP, K)
    lg = nlab.reshape(NP, K)
    maxp = np.maximum(np.max(np.where(lg == 0, sg, -np.inf), axis=1), -1.0)
    minp = np.minimum(np.min(np.where(lg == 1, sg, np.inf), axis=1), 1.0)

    # ---- device kernel 2: dots of gathered hard-positive columns ---------
    run2 = _get_runner("k2")
    in_maps2 = []
    for c in range(NCORES):
        # col j = k*NPL + p_local -> global nlp id 4*furthest[p] + k
        fth = furthest[c * NPL:(c + 1) * NPL]
        cols = (np.arange(K)[:, None] + fth[None, :] * K).reshape(-1)
        in_maps2.append({
            "x8": _x_dev(X8, cols),
            "po8x": po_dev[c],
        })
    res2 = run2(in_maps2)
    cur = _scores_from_dev(res2).reshape(NP, K)

    nids = (furthest[:, None] * K + np.arange(K)).reshape(-1)           # [NN]
    lcur = nlab[nids].reshape(NP, K)
    maxcur = np.max(np.where(lcur == 1, cur, -np.inf), axis=1)
    maxp = np.maximum(maxp, maxcur)
    found = ~((maxp == -1.0) | (minp == 1.0))
    lt = np.where(found, maxp - minp + 2.0, 0.0).astype(np.float32)
    not_find = int(np.sum(~found))
    if not_find == NN:
        loss_triple = np.float32(0.0)
    else:
        loss_triple = np.float32(lt.sum(dtype=np.float32) / np.float32(NN - not_find))

    return (np.float32(loss_label), np.float32(loss_norm), np.float32(loss_triple))


# revision 24
# speedup vs baseline: 2.8810x; 1.1452x over previous
"""Trainium2 Bass kernel for nn_ContrastLoss_Disentangle.

Contract: kernel(**inputs) takes the FULL (unsharded) inputs and returns
(loss_label, loss_norm, loss_triple) as float32 scalars.

Pipeline (8 NeuronCores, data-parallel over pose/nlp rows):
  host:    norms (pose+nlp), normalization, X = nlpF * categories / norm
           (bf16, cd-partition interleaved layout), poseFT fp8 (x16) for the
           gram matrix, poseFT bf16 interleaved for the dot products
  device1: per-core: scores for the core's 1024 nlp rows (DVE bf16 mult +
           PE ones-matmul partition reduction) and a [512, 1024] block of
           the pose gram matrix (PE, fp8 DoubleRow), emitted as f16
  host:    BCE, pm assembly + stable argsort rank-select (furthest), gather
           of the hard-positive X columns
  device2: per-core: cur dots of the gathered X columns (same structure)
  host:    triplet loss assembly
"""

import os
import numpy as np
import ml_dtypes

import concourse.bass as bass
import concourse.tile as tile
from concourse import bacc, mybir
from concourse.bass2jax import install_neuronx_cc_hook, partition_id_tensor, _bass_exec_p

C, NP, K, D = 8, 2048, 4, 256
NN = NP * K          # 8192
NCORES = 8
NPL = NP // NCORES   # 256 poses per core
NNL = NN * 1 // NCORES  # 1024 nlp rows per core
CD = C * D           # 2048 contraction size
KC = CD // 128       # 16 cd chunks of 128 partitions

# pm block grid: 4 row-blocks x 2 col-blocks
PM_MI, PM_NJ = 4, 2
PM_M = NP // PM_MI   # 512 rows per core block
PM_N = NP // PM_NJ   # 1024 cols per core block
KT2 = CD // 256      # 8 DoubleRow contraction chunks

PM_SCALE = 16.0      # fp8 pre-scale for pose columns (argsort is scale-invariant)
X_SCALE = 8.0        # fp8 pre-scale for X columns (scores come back x128)

BF16 = ml_dtypes.bfloat16
FP8 = ml_dtypes.float8_e4m3

_runners = {}


NM = 8  # m-chunks of 128 score columns per core


def _build_kernel(with_pm: bool):
    """Per-core program. Inputs (per core):
      x8  [128, NM, KT2, 2, 128] fp8  X columns (x8 scale), m-packed:
          x8[p, m, kc2, t, jj] = X8[kc2*256+t*128+p, col(m*128+jj)]
          with col() the core's (k, pose)-major order
      po8 [128, KT2, 2, NPL] fp8  pose columns (x16 scale), same packing
      msk [128, 2, 256] bf16  diagonal masks: msk[jj, par, q] = (q==jj+128*par)
      pml [CD, PM_M] fp8, pmr [CD, PM_N] fp8  (only when with_pm)
    Outputs:
      sc  [128, NM] f32   128*score[m*128+jj] at [jj, m]
      pmblk [PM_M, PM_N] f16  (only when with_pm)
    """
    f8 = mybir.dt.float8e4
    f16 = mybir.dt.float16
    f32 = mybir.dt.float32
    nc = bacc.Bacc("TRN2", target_bir_lowering=False, debug=False,
                   num_devices=NCORES)
    x8 = nc.dram_tensor("x8", [128, NM, KT2, 2, 128], f8,
                        kind="ExternalInput").ap()
    # po8x: slots 0..KT2-1 = pose columns; slot KT2 = diagonal masks
    po8x = nc.dram_tensor("po8x", [128, KT2 + 1, 2, NPL], f8,
                          kind="ExternalInput").ap()
    if with_pm:
        pml = nc.dram_tensor("pml", [CD, PM_M], f8, kind="ExternalInput").ap()
        pmr = nc.dram_tensor("pmr", [CD, PM_N], f8, kind="ExternalInput").ap()
        pmblk = nc.dram_tensor("pmblk", [PM_M, PM_N], f16,
                               kind="ExternalOutput").ap()
    sc = nc.dram_tensor("sc", [128, NM], f16, kind="ExternalOutput").ap()

    DR = mybir.MatmulPerfMode.DoubleRow
    with tile.TileContext(nc) as tc:
        with tc.tile_pool(name="cst", bufs=1) as cst, \
             tc.tile_pool(name="zp", bufs=3) as zp, \
             tc.tile_pool(name="ev", bufs=2) as ev, \
             tc.tile_pool(name="ps", bufs=2, space="PSUM") as ps, \
             tc.tile_pool(name="ps_s", bufs=2, space="PSUM") as ps_s:

            po8_t = cst.tile([128, KT2 + 1, 2, NPL], f8, tag="po8")
            x8_t = cst.tile([128, NM, KT2, 2, 128], f8, tag="x8")
            sct = cst.tile([128, NM], f16, tag="sct")

            def po_load():
                nc.sync.dma_start(po8_t[:], po8x)

            def x_load(ci, nch=1):
                nc.sync.dma_start(x8_t[:, ci:ci + nch], x8[:, ci:ci + nch])

            def scores_m(m):
                psc = ps_s.tile([128, 256], f32, tag="psc", name=f"psc{m}")
                for kc2 in range(KT2):
                    nc.tensor.matmul(psc[:], x8_t[:, m, kc2],
                                     po8_t[:, kc2][:],
                                     start=(kc2 == 0), stop=(kc2 == KT2 - 1),
                                     perf_mode=DR, skip_group_check=True)
                zm = zp.tile([128, 256], f16, tag="zm", name=f"zm{m}")
                with nc.allow_low_precision(reason="f16 scores ok"):
                    if os.environ.get("NO_TTR"):
                        nc.vector.tensor_tensor(zm[:], psc[:],
                                                po8_t[:, KT2, m & 1],
                                                op=mybir.AluOpType.mult)
                        nc.vector.tensor_reduce(sct[:, m:m + 1], zm[:],
                                                axis=mybir.AxisListType.X,
                                                op=mybir.AluOpType.add)
                    else:
                        nc.vector.tensor_tensor_reduce(
                            zm[:], psc[:], po8_t[:, KT2, m & 1], scale=1.0,
                            scalar=0.0, op0=mybir.AluOpType.mult,
                            op1=mybir.AluOpType.add, accum_out=sct[:, m:m + 1])

            def s_flush():
                nc.scalar.dma_start(sc, sct[:])

            # ---------- pm sub-program ------------------------------------
            if with_pm:
                lt = cst.tile([128, KT2, 2, PM_M], f8, tag="lt")
                rt = cst.tile([128, KT2, 2, PM_N], f8, tag="rt")

                def lt_load(half):
                    nc.sync.dma_start(
                        lt[:, 4 * half:4 * (half + 1)],
                        pml[1024 * half:1024 * (half + 1), :]
                        .rearrange("(kc t p) m -> p kc t m", p=128, t=2))

                def rt_load(n, half):
                    nc.sync.dma_start(
                        rt[:, 4 * half:4 * (half + 1), :,
                           512 * n:512 * (n + 1)],
                        pmr.rearrange("(kc t p) q -> p kc t q", p=128, t=2)
                        [:, 4 * half:4 * (half + 1), :,
                         512 * n:512 * (n + 1)])

                evs = {}

                def pm_mchain(n, m):
                    acc = ps.tile([128, 512], f32, tag="acc",
                                  name=f"acc{n}{m}")
                    for kc in range(KT2):
                        nc.tensor.matmul(acc[:],
                                         lt[:, kc, :, 128 * m:128 * (m + 1)],
                                         rt[:, kc, :, 512 * n:512 * (n + 1)],
                                         start=(kc == 0),
                                         stop=(kc == KT2 - 1),
                                         perf_mode=DR, skip_group_check=True)
                    if m == 0:
                        evs[n] = ev.tile([128, 4, 512], f16, tag="ev",
                                         name=f"ev{n}")
                    nc.scalar.copy(evs[n][:, m][:], acc[:])
                    if m == PM_MI - 1:
                        nc.scalar.dma_start(
                            pmblk.rearrange("(m p) q -> p m q", p=128)
                            [:, :, 512 * n:512 * (n + 1)], evs[n][:])

            # ---------- emission order ------------------------------------
            if with_pm:
                lt_load(0)
                rt_load(0, 0)
                lt_load(1)
                rt_load(0, 1)
                pm_mchain(0, 0)
                rt_load(1, 0)
                rt_load(1, 1)
                pm_mchain(0, 1)
                po_load()
                pm_mchain(0, 2)
                x_load(0, 2)
                pm_mchain(0, 3)
                pm_mchain(1, 0)
                x_load(2, 2)
                pm_mchain(1, 1)
                scores_m(0)
                pm_mchain(1, 2)
                scores_m(1)
                x_load(4, 2)
                pm_mchain(1, 3)
                scores_m(2)
                scores_m(3)
                x_load(6, 2)
                scores_m(4)
                scores_m(5)
                scores_m(6)
                scores_m(7)
                s_flush()
            else:
                po_load()
                for m in range(NM):
                    x_load(m)
                    scores_m(m)
                s_flush()

    nc.finalize()
    return nc


def _make_runner(nc):
    """Reusable jitted SPMD runner (replicates bass2jax.run_bass_via_pjrt but
    caches the compiled executable across calls)."""
    import jax
    from jax.sharding import Mesh, PartitionSpec
    from jax.experimental.shard_map import shard_map

    install_neuronx_cc_hook()
    partition_name = nc.partition_id_tensor.name if nc.partition_id_tensor else None
    in_names, out_names, out_avals = [], [], []
    for alloc in nc.m.functions[0].allocations:
        if not isinstance(alloc, mybir.MemoryLocationSet):
            continue
        name = alloc.memorylocations[0].name
        if alloc.kind == "ExternalInput":
            if name != partition_name:
                in_names.append(name)
        elif alloc.kind == "ExternalOutput":
            out_names.append(name)
            out_avals.append(jax.core.ShapedArray(
                tuple(alloc.tensor_shape), mybir.dt.np(alloc.dtype)))
    n_params = len(in_names)
    all_in = in_names + out_names + ([partition_name] if partition_name else [])

    def _body(*args):
        operands = list(args)
        if partition_name is not None:
            operands.append(partition_id_tensor())
        outs = _bass_exec_p.bind(
            *operands, out_avals=tuple(out_avals), in_names=tuple(all_in),
            out_names=tuple(out_names), lowering_input_output_aliases=(),
            sim_require_finite=False, sim_require_nnan=False, nc=nc)
        return tuple(outs)

    devices = jax.devices()[:NCORES]
    mesh = Mesh(np.asarray(devices), ("core",))
    donate = tuple(range(n_params, n_params + len(out_names)))
    sharded = jax.jit(
        shard_map(_body, mesh=mesh,
                  in_specs=(PartitionSpec("core"),) * (n_params + len(out_names)),
                  out_specs=(PartitionSpec("core"),) * len(out_names),
                  check_rep=False),
        donate_argnums=donate, keep_unused=True)

    def run(in_maps):
        concat_in = [np.concatenate([np.asarray(m[name]) for m in in_maps], axis=0)
                     for name in in_names]
        zeros = [np.zeros((NCORES * a.shape[0], *a.shape[1:]), a.dtype)
                 for a in out_avals]
        out_arrs = sharded(*concat_in, *zeros)
        return [
            {name: np.asarray(out_arrs[i]).reshape(NCORES, *out_avals[i].shape)[c]
             for i, name in enumerate(out_names)}
            for c in range(NCORES)
        ]

    return run


def _get_runner(key):
    if key not in _runners:
        _runners[key] = _make_runner(_build_kernel(with_pm=(key == "k1")))
    return _runners[key]


def _x_dev(X8, cols):
    """Columns `cols` (global nlp ids, (k, pose)-major order per core) of the
    fp8 [CD, NN] X^T matrix -> [128, NM, KT2, 2, 128] device layout."""
    A = X8[:, cols]                                # [CD, 1024]
    return np.ascontiguousarray(
        A.reshape(KT2, 2, 128, NM, 128).transpose(2, 3, 0, 1, 4))


def _po_dev(P8c, mskv):
    """[CD, NPL] fp8 pose columns -> [128, KT2+1, 2, NPL] device layout with
    the diagonal masks appended as slot KT2."""
    po = P8c.reshape(KT2, 2, 128, NPL).transpose(2, 0, 1, 3)
    return np.ascontiguousarray(
        np.concatenate([po, mskv[:, None, :, :]], axis=1))


def _scores_from_dev(res, name="sc"):
    """Per-core [128, NM] (row jj, col m -> j = m*128+jj, scaled x128, with
    col j = k*NPL + p_local) -> [NN] global scores."""
    out = np.empty((NCORES, NPL, K), np.float32)
    for c in range(NCORES):
        sc = np.asarray(res[c][name], np.float32)
        out[c] = (sc.T.reshape(K * NPL) / 128.0).reshape(K, NPL).T
    return out.reshape(NN)


def _kernel_host_fallback(inputs):
    """Pure-numpy reference replication, used only if the index tensors do
    not have the canonical arange structure the device layout relies on."""
    nlp = np.asarray(inputs["nlp_features"], np.float32)
    pose = np.asarray(inputs["pose_features"], np.float32)
    nlab = np.asarray(inputs["nlp_label"]).astype(np.int64)
    n2p = np.asarray(inputs["nlpid2poseid"]).astype(np.int64)
    p2n = np.asarray(inputs["pose2nlpid"]).astype(np.int64)
    cat = np.asarray(inputs["categories"], np.float32)
    ri = np.asarray(inputs["rand_index"]).astype(np.int64)
    Np, Nn = pose.shape[1], nlp.shape[1]
    norm_p = np.sqrt(np.einsum("cpd,cpd->cp", pose, pose, dtype=np.float32))
    norm_n = np.sqrt(np.einsum("cnd,cnd->cn", nlp, nlp, dtype=np.float32))
    poseF = pose / norm_p[:, :, None]
    nlpF = nlp / norm_n[:, :, None]
    loss_norm = np.float32(np.float32(norm_p.mean()) + np.float32(norm_n.mean()))
    dots = np.einsum("cnd,cnd->cn", nlpF, poseF[:, n2p]).astype(np.float32)
    scores = np.einsum("cn,nc->n", dots, cat).astype(np.float32)
    p = (1.0 / (1.0 + np.exp(-scores))).astype(np.float32)
    lblf = nlab.astype(np.float32)
    loss_label = np.float32(
        np.mean(-(np.log(p) * lblf + np.log(1.0 - p) * (1.0 - lblf))))
    pf = np.ascontiguousarray(poseF.transpose(0, 2, 1).reshape(-1, Np))
    pm = (pf.T @ pf).astype(np.float32)
    ar = np.arange(Np)
    pm[ar, ar] = 1.0
    order = np.argsort(pm, axis=1, kind="stable")
    furthest = order[ar, ri]
    sg = scores[p2n]
    lg = nlab[p2n]
    maxp = np.maximum(np.max(np.where(lg == 0, sg, -np.inf), axis=1), -1.0)
    minp = np.minimum(np.min(np.where(lg == 1, sg, np.inf), axis=1), 1.0)
    nids = p2n[furthest]
    cd = np.einsum("cpkd,cpd->cpk", nlpF[:, nids], poseF)
    cur = np.einsum("cpk,pkc->pk", cd, cat[nids]).astype(np.float32)
    lcur = nlab[nids]
    maxcur = np.max(np.where(lcur == 1, cur, -np.inf), axis=1)
    maxp = np.maximum(maxp, maxcur)
    found = ~((maxp == -1.0) | (minp == 1.0))
    lt = np.where(found, maxp - minp + 2.0, 0.0).astype(np.float32)
    nf = int(np.sum(~found))
    loss_triple = (np.float32(0.0) if nf == Nn else
                   np.float32(lt.sum(dtype=np.float32) / np.float32(Nn - nf)))
    return (np.float32(loss_label), loss_norm, loss_triple)


def kernel(**inputs):
    nlp = np.ascontiguousarray(inputs["nlp_features"], np.float32)      # [C, NN, D]
    pose = np.ascontiguousarray(inputs["pose_features"], np.float32)    # [C, NP, D]
    nlab = np.asarray(inputs["nlp_label"]).astype(np.int64)
    cat = np.ascontiguousarray(inputs["categories"], np.float32)        # [NN, C]
    ri = np.asarray(inputs["rand_index"]).astype(np.int64)

    n2p = np.asarray(inputs["nlpid2poseid"]).astype(np.int64)
    p2n = np.asarray(inputs["pose2nlpid"]).astype(np.int64)
    if (not np.array_equal(n2p, np.arange(NN) // K)
            or not np.array_equal(p2n, np.arange(NN).reshape(NP, K))):
        return _kernel_host_fallback(inputs)

    # ---- host: norms, X^T (bf16), poseFT (bf16 + fp8) --------------------
    norm_p = np.sqrt(np.einsum("cpd,cpd->cp", pose, pose, dtype=np.float32,
                               optimize=True)).astype(np.float32)       # [C, NP]
    norm_n = np.sqrt(np.einsum("cnd,cnd->cn", nlp, nlp, dtype=np.float32,
                               optimize=True)).astype(np.float32)       # [C, NN]
    loss_norm = np.float32(np.float32(norm_p.mean()) + np.float32(norm_n.mean()))

    poseF = pose / norm_p[:, :, None]
    poseFT = np.ascontiguousarray(
        poseF.transpose(0, 2, 1).reshape(CD, NP))                       # [CD, NP]
    P8 = (poseFT * PM_SCALE).astype(FP8)                                # [CD, NP]

    scale = (cat.T / norm_n).astype(np.float32)                         # [C, NN]
    X8 = ((nlp * scale[:, :, None]).transpose(0, 2, 1).reshape(CD, NN)
          * X_SCALE).astype(FP8)                                        # [CD, NN]

    # (k, pose)-major column order within each core's 1024 nlp columns
    base_cols = (np.arange(NPL)[None, :] * K + np.arange(K)[:, None]).reshape(-1)

    mskv = np.zeros((128, 2, 256), FP8)
    jj = np.arange(128)
    mskv[jj, 0, jj] = 1
    mskv[jj, 1, jj + 128] = 1

    # ---- device kernel 1: scores + pm blocks -----------------------------
    run1 = _get_runner("k1")
    po_dev = [_po_dev(np.ascontiguousarray(P8[:, c * NPL:(c + 1) * NPL]), mskv)
              for c in range(NCORES)]
    in_maps = []
    for c in range(NCORES):
        i, j = c // PM_NJ, c % PM_NJ
        in_maps.append({
            "x8": _x_dev(X8, c * NNL + base_cols),
            "po8x": po_dev[c],
            "pml": np.ascontiguousarray(P8[:, i * PM_M:(i + 1) * PM_M]),
            "pmr": np.ascontiguousarray(P8[:, j * PM_N:(j + 1) * PM_N]),
        })
    res1 = run1(in_maps)

    scores = _scores_from_dev(res1)                                     # [NN]
    pm = np.empty((NP, NP), np.float32)
    for c in range(NCORES):
        i, j = c // PM_NJ, c % PM_NJ
        pm[i * PM_M:(i + 1) * PM_M, j * PM_N:(j + 1) * PM_N] = \
            res1[c]["pmblk"].astype(np.float32)

    # ---- host: BCE -------------------------------------------------------
    p = (1.0 / (1.0 + np.exp(-scores))).astype(np.float32)
    lblf = nlab.astype(np.float32)
    loss_label = np.float32(
        np.mean(-(np.log(p) * lblf + np.log(1.0 - p) * (1.0 - lblf))))

    # ---- host: furthest selection (pm is PM_SCALE^2 * gram) --------------
    ar = np.arange(NP)
    pm[ar, ar] = PM_SCALE * PM_SCALE
    order = np.argsort(pm, axis=1, kind="stable")
    furthest = order[ar, ri]                                            # [NP]

    sg = scores.reshape(NP, K)
    lg = nlab.reshape(NP, K)
    maxp = np.maximum(np.max(np.where(lg == 0, sg, -np.inf), axis=1), -1.0)
    minp = np.minimum(np.min(np.where(lg == 1, sg, np.inf), axis=1), 1.0)

    # ---- device kernel 2: dots of gathered hard-positive columns ---------
    run2 = _get_runner("k2")
    in_maps2 = []
    for c in range(NCORES):
        # col j = k*NPL + p_local -> global nlp id 4*furthest[p] + k
        fth = furthest[c * NPL:(c + 1) * NPL]
        cols = (np.arange(K)[:, None] + fth[None, :] * K).reshape(-1)
        in_maps2.append({
            "x8": _x_dev(X8, cols),
            "po8x": po_dev[c],
        })
    res2 = run2(in_maps2)
    cur = _scores_from_dev(res2).reshape(NP, K)

    nids = (furthest[:, None] * K + np.arange(K)).reshape(-1)           # [NN]
    lcur = nlab[nids].reshape(NP, K)
    maxcur = np.max(np.where(lcur == 1, cur, -np.inf), axis=1)
    maxp = np.maximum(maxp, maxcur)
    found = ~((maxp == -1.0) | (minp == 1.0))
    lt = np.where(found, maxp - minp + 2.0, 0.0).astype(np.float32)
    not_find = int(np.sum(~found))
    if not_find == NN:
        loss_triple = np.float32(0.0)
    else:
        loss_triple = np.float32(lt.sum(dtype=np.float32) / np.float32(NN - not_find))

    return (np.float32(loss_label), np.float32(loss_norm), np.float32(loss_triple))


# revision 25
# speedup vs baseline: 3.0569x; 1.0610x over previous
"""Trainium2 Bass kernel for nn_ContrastLoss_Disentangle.

Contract: kernel(**inputs) takes the FULL (unsharded) inputs and returns
(loss_label, loss_norm, loss_triple) as float32 scalars.

Pipeline (8 NeuronCores, data-parallel over pose/nlp rows):
  host:    norms (pose+nlp), normalization, X = nlpF * categories / norm
           (bf16, cd-partition interleaved layout), poseFT fp8 (x16) for the
           gram matrix, poseFT bf16 interleaved for the dot products
  device1: per-core: scores for the core's 1024 nlp rows (DVE bf16 mult +
           PE ones-matmul partition reduction) and a [512, 1024] block of
           the pose gram matrix (PE, fp8 DoubleRow), emitted as f16
  host:    BCE, pm assembly + stable argsort rank-select (furthest), gather
           of the hard-positive X columns
  device2: per-core: cur dots of the gathered X columns (same structure)
  host:    triplet loss assembly
"""

import os
import numpy as np
import ml_dtypes

import concourse.bass as bass
import concourse.tile as tile
from concourse import bacc, mybir
from concourse.bass2jax import install_neuronx_cc_hook, partition_id_tensor, _bass_exec_p

C, NP, K, D = 8, 2048, 4, 256
NN = NP * K          # 8192
NCORES = 8
NPL = NP // NCORES   # 256 poses per core
NNL = NN * 1 // NCORES  # 1024 nlp rows per core
CD = C * D           # 2048 contraction size
KC = CD // 128       # 16 cd chunks of 128 partitions

# pm block grid: 4 row-blocks x 2 col-blocks
PM_MI, PM_NJ = 4, 2
PM_M = NP // PM_MI   # 512 rows per core block
PM_N = NP // PM_NJ   # 1024 cols per core block
KT2 = CD // 256      # 8 DoubleRow contraction chunks

PM_SCALE = 16.0      # fp8 pre-scale for pose columns (argsort is scale-invariant)
X_SCALE = 8.0        # fp8 pre-scale for X columns (scores come back x128)

BF16 = ml_dtypes.bfloat16
FP8 = ml_dtypes.float8_e4m3

_runners = {}


NM = 8  # m-chunks of 128 score columns per core


def _build_kernel(with_pm: bool):
    """Per-core program. Inputs (per core):
      x8  [128, NM, KT2, 2, 128] fp8  X columns (x8 scale), m-packed:
          x8[p, m, kc2, t, jj] = X8[kc2*256+t*128+p, col(m*128+jj)]
          with col() the core's (k, pose)-major order
      po8 [128, KT2, 2, NPL] fp8  pose columns (x16 scale), same packing
      msk [128, 2, 256] bf16  diagonal masks: msk[jj, par, q] = (q==jj+128*par)
      pml [CD, PM_M] fp8, pmr [CD, PM_N] fp8  (only when with_pm)
    Outputs:
      sc  [128, NM] f32   128*score[m*128+jj] at [jj, m]
      pmblk [PM_M, PM_N] f16  (only when with_pm)
    """
    f8 = mybir.dt.float8e4
    f16 = mybir.dt.float16
    f32 = mybir.dt.float32
    nc = bacc.Bacc("TRN2", target_bir_lowering=False, debug=False,
                   num_devices=NCORES)
    x8 = nc.dram_tensor("x8", [128, NM, KT2, 2, 128], f8,
                        kind="ExternalInput").ap()
    # po8x: slots 0..KT2-1 = pose columns; slot KT2 = diagonal masks
    po8x = nc.dram_tensor("po8x", [128, KT2 + 1, 2, NPL], f8,
                          kind="ExternalInput").ap()
    if with_pm:
        pml = nc.dram_tensor("pml", [CD, PM_M], f8, kind="ExternalInput").ap()
        pmr = nc.dram_tensor("pmr", [CD, PM_N], f8, kind="ExternalInput").ap()
        pmblk = nc.dram_tensor("pmblk", [PM_M, PM_N], f16,
                               kind="ExternalOutput").ap()
    sc = nc.dram_tensor("sc", [128, NM], f16, kind="ExternalOutput").ap()

    DR = mybir.MatmulPerfMode.DoubleRow
    with tile.TileContext(nc) as tc:
        with tc.tile_pool(name="cst", bufs=1) as cst, \
             tc.tile_pool(name="zp", bufs=3) as zp, \
             tc.tile_pool(name="ev", bufs=2) as ev, \
             tc.tile_pool(name="ps", bufs=2, space="PSUM") as ps, \
             tc.tile_pool(name="ps_s", bufs=2, space="PSUM") as ps_s:

            po8_t = cst.tile([128, KT2 + 1, 2, NPL], f8, tag="po8")
            x8_t = cst.tile([128, NM, KT2, 2, 128], f8, tag="x8")
            sct = cst.tile([128, NM], f16, tag="sct")

            def po_load():
                nc.sync.dma_start(po8_t[:], po8x)

            def x_load(ci, nch=1):
                nc.sync.dma_start(x8_t[:, ci:ci + nch], x8[:, ci:ci + nch])

            def scores_m(m):
                psc = ps_s.tile([128, 256], f32, tag="psc", name=f"psc{m}")
                for kc2 in range(KT2):
                    nc.tensor.matmul(psc[:], x8_t[:, m, kc2],
                                     po8_t[:, kc2][:],
                                     start=(kc2 == 0), stop=(kc2 == KT2 - 1),
                                     perf_mode=DR, skip_group_check=True)
                zm = zp.tile([128, 256], f16, tag="zm", name=f"zm{m}")
                with nc.allow_low_precision(reason="f16 scores ok"):
                    mode = os.environ.get("EXTRACT", "stt")
                    if mode == "2op":
                        nc.vector.tensor_tensor(zm[:], psc[:],
                                                po8_t[:, KT2, m & 1],
                                                op=mybir.AluOpType.mult)
                        nc.vector.tensor_reduce(sct[:, m:m + 1], zm[:],
                                                axis=mybir.AxisListType.X,
                                                op=mybir.AluOpType.add)
                    elif mode == "stt":
                        nc.vector.scalar_tensor_tensor(
                            zm[:], psc[:], 1.0, po8_t[:, KT2, m & 1],
                            op0=mybir.AluOpType.mult,
                            op1=mybir.AluOpType.mult,
                            accum_out=sct[:, m:m + 1])
                    else:
                        nc.vector.tensor_tensor_reduce(
                            zm[:], psc[:], po8_t[:, KT2, m & 1], scale=1.0,
                            scalar=0.0, op0=mybir.AluOpType.mult,
                            op1=mybir.AluOpType.add, accum_out=sct[:, m:m + 1])

            def s_flush():
                nc.scalar.dma_start(sc, sct[:])

            # ---------- pm sub-program ------------------------------------
            if with_pm:
                lt = cst.tile([128, KT2, 2, PM_M], f8, tag="lt")
                rt = cst.tile([128, KT2, 2, PM_N], f8, tag="rt")

                def lt_load(half):
                    nc.sync.dma_start(
                        lt[:, 4 * half:4 * (half + 1)],
                        pml[1024 * half:1024 * (half + 1), :]
                        .rearrange("(kc t p) m -> p kc t m", p=128, t=2))

                def rt_load(n, half):
                    nc.sync.dma_start(
                        rt[:, 4 * half:4 * (half + 1), :,
                           512 * n:512 * (n + 1)],
                        pmr.rearrange("(kc t p) q -> p kc t q", p=128, t=2)
                        [:, 4 * half:4 * (half + 1), :,
                         512 * n:512 * (n + 1)])

                evs = {}

                def pm_mchain(n, m):
                    acc = ps.tile([128, 512], f32, tag="acc",
                                  name=f"acc{n}{m}")
                    for kc in range(KT2):
                        nc.tensor.matmul(acc[:],
                                         lt[:, kc, :, 128 * m:128 * (m + 1)],
                                         rt[:, kc, :, 512 * n:512 * (n + 1)],
                                         start=(kc == 0),
                                         stop=(kc == KT2 - 1),
                                         perf_mode=DR, skip_group_check=True)
                    if m == 0:
                        evs[n] = ev.tile([128, 4, 512], f16, tag="ev",
                                         name=f"ev{n}")
                    nc.scalar.copy(evs[n][:, m][:], acc[:])
                    if m == PM_MI - 1:
                        nc.scalar.dma_start(
                            pmblk.rearrange("(m p) q -> p m q", p=128)
                            [:, :, 512 * n:512 * (n + 1)], evs[n][:])

            # ---------- emission order ------------------------------------
            if with_pm:
                lt_load(0)
                rt_load(0, 0)
                lt_load(1)
                rt_load(0, 1)
                pm_mchain(0, 0)
                rt_load(1, 0)
                rt_load(1, 1)
                pm_mchain(0, 1)
                po_load()
                pm_mchain(0, 2)
                x_load(0, 2)
                pm_mchain(0, 3)
                pm_mchain(1, 0)
                x_load(2, 2)
                pm_mchain(1, 1)
                scores_m(0)
                pm_mchain(1, 2)
                scores_m(1)
                x_load(4, 2)
                pm_mchain(1, 3)
                scores_m(2)
                scores_m(3)
                x_load(6, 2)
                scores_m(4)
                scores_m(5)
                scores_m(6)
                scores_m(7)
                s_flush()
            else:
                po_load()
                for m in range(NM):
                    x_load(m)
                    scores_m(m)
                s_flush()

    nc.finalize()
    return nc


def _make_runner(nc):
    """Reusable jitted SPMD runner (replicates bass2jax.run_bass_via_pjrt but
    caches the compiled executable across calls)."""
    import jax
    from jax.sharding import Mesh, PartitionSpec
    from jax.experimental.shard_map import shard_map

    install_neuronx_cc_hook()
    partition_name = nc.partition_id_tensor.name if nc.partition_id_tensor else None
    in_names, out_names, out_avals = [], [], []
    for alloc in nc.m.functions[0].allocations:
        if not isinstance(alloc, mybir.MemoryLocationSet):
            continue
        name = alloc.memorylocations[0].name
        if alloc.kind == "ExternalInput":
            if name != partition_name:
                in_names.append(name)
        elif alloc.kind == "ExternalOutput":
            out_names.append(name)
            out_avals.append(jax.core.ShapedArray(
                tuple(alloc.tensor_shape), mybir.dt.np(alloc.dtype)))
    n_params = len(in_names)
    all_in = in_names + out_names + ([partition_name] if partition_name else [])

    def _body(*args):
        operands = list(args)
        if partition_name is not None:
            operands.append(partition_id_tensor())
        outs = _bass_exec_p.bind(
            *operands, out_avals=tuple(out_avals), in_names=tuple(all_in),
            out_names=tuple(out_names), lowering_input_output_aliases=(),
            sim_require_finite=False, sim_require_nnan=False, nc=nc)
        return tuple(outs)

    devices = jax.devices()[:NCORES]
    mesh = Mesh(np.asarray(devices), ("core",))
    donate = tuple(range(n_params, n_params + len(out_names)))
    sharded = jax.jit(
        shard_map(_body, mesh=mesh,
                  in_specs=(PartitionSpec("core"),) * (n_params + len(out_names)),
                  out_specs=(PartitionSpec("core"),) * len(out_names),
                  check_rep=False),
        donate_argnums=donate, keep_unused=True)

    def run(in_maps):
        concat_in = [np.concatenate([np.asarray(m[name]) for m in in_maps], axis=0)
                     for name in in_names]
        zeros = [np.zeros((NCORES * a.shape[0], *a.shape[1:]), a.dtype)
                 for a in out_avals]
        out_arrs = sharded(*concat_in, *zeros)
        return [
            {name: np.asarray(out_arrs[i]).reshape(NCORES, *out_avals[i].shape)[c]
             for i, name in enumerate(out_names)}
            for c in range(NCORES)
        ]

    return run


def _get_runner(key):
    if key not in _runners:
        _runners[key] = _make_runner(_build_kernel(with_pm=(key == "k1")))
    return _runners[key]


def _x_dev(X8, cols):
    """Columns `cols` (global nlp ids, (k, pose)-major order per core) of the
    fp8 [CD, NN] X^T matrix -> [128, NM, KT2, 2, 128] device layout."""
    A = X8[:, cols]                                # [CD, 1024]
    return np.ascontiguousarray(
        A.reshape(KT2, 2, 128, NM, 128).transpose(2, 3, 0, 1, 4))


def _po_dev(P8c, mskv):
    """[CD, NPL] fp8 pose columns -> [128, KT2+1, 2, NPL] device layout with
    the diagonal masks appended as slot KT2."""
    po = P8c.reshape(KT2, 2, 128, NPL).transpose(2, 0, 1, 3)
    return np.ascontiguousarray(
        np.concatenate([po, mskv[:, None, :, :]], axis=1))


def _scores_from_dev(res, name="sc"):
    """Per-core [128, NM] (row jj, col m -> j = m*128+jj, scaled x128, with
    col j = k*NPL + p_local) -> [NN] global scores."""
    out = np.empty((NCORES, NPL, K), np.float32)
    for c in range(NCORES):
        sc = np.asarray(res[c][name], np.float32)
        out[c] = (sc.T.reshape(K * NPL) / 128.0).reshape(K, NPL).T
    return out.reshape(NN)


def _kernel_host_fallback(inputs):
    """Pure-numpy reference replication, used only if the index tensors do
    not have the canonical arange structure the device layout relies on."""
    nlp = np.asarray(inputs["nlp_features"], np.float32)
    pose = np.asarray(inputs["pose_features"], np.float32)
    nlab = np.asarray(inputs["nlp_label"]).astype(np.int64)
    n2p = np.asarray(inputs["nlpid2poseid"]).astype(np.int64)
    p2n = np.asarray(inputs["pose2nlpid"]).astype(np.int64)
    cat = np.asarray(inputs["categories"], np.float32)
    ri = np.asarray(inputs["rand_index"]).astype(np.int64)
    Np, Nn = pose.shape[1], nlp.shape[1]
    norm_p = np.sqrt(np.einsum("cpd,cpd->cp", pose, pose, dtype=np.float32))
    norm_n = np.sqrt(np.einsum("cnd,cnd->cn", nlp, nlp, dtype=np.float32))
    poseF = pose / norm_p[:, :, None]
    nlpF = nlp / norm_n[:, :, None]
    loss_norm = np.float32(np.float32(norm_p.mean()) + np.float32(norm_n.mean()))
    dots = np.einsum("cnd,cnd->cn", nlpF, poseF[:, n2p]).astype(np.float32)
    scores = np.einsum("cn,nc->n", dots, cat).astype(np.float32)
    p = (1.0 / (1.0 + np.exp(-scores))).astype(np.float32)
    lblf = nlab.astype(np.float32)
    loss_label = np.float32(
        np.mean(-(np.log(p) * lblf + np.log(1.0 - p) * (1.0 - lblf))))
    pf = np.ascontiguousarray(poseF.transpose(0, 2, 1).reshape(-1, Np))
    pm = (pf.T @ pf).astype(np.float32)
    ar = np.arange(Np)
    pm[ar, ar] = 1.0
    order = np.argsort(pm, axis=1, kind="stable")
    furthest = order[ar, ri]
    sg = scores[p2n]
    lg = nlab[p2n]
    maxp = np.maximum(np.max(np.where(lg == 0, sg, -np.inf), axis=1), -1.0)
    minp = np.minimum(np.min(np.where(lg == 1, sg, np.inf), axis=1), 1.0)
    nids = p2n[furthest]
    cd = np.einsum("cpkd,cpd->cpk", nlpF[:, nids], poseF)
    cur = np.einsum("cpk,pkc->pk", cd, cat[nids]).astype(np.float32)
    lcur = nlab[nids]
    maxcur = np.max(np.where(lcur == 1, cur, -np.inf), axis=1)
    maxp = np.maximum(maxp, maxcur)
    found = ~((maxp == -1.0) | (minp == 1.0))
    lt = np.where(found, maxp - minp + 2.0, 0.0).astype(np.float32)
    nf = int(np.sum(~found))
    loss_triple = (np.float32(0.0) if nf == Nn else
                   np.float32(lt.sum(dtype=np.float32) / np.float32(Nn - nf)))
    return (np.float32(loss_label), loss_norm, loss_triple)


def kernel(**inputs):
    nlp = np.ascontiguousarray(inputs["nlp_features"], np.float32)      # [C, NN, D]
    pose = np.ascontiguousarray(inputs["pose_features"], np.float32)    # [C, NP, D]
    nlab = np.asarray(inputs["nlp_label"]).astype(np.int64)
    cat = np.ascontiguousarray(inputs["categories"], np.float32)        # [NN, C]
    ri = np.asarray(inputs["rand_index"]).astype(np.int64)

    n2p = np.asarray(inputs["nlpid2poseid"]).astype(np.int64)
    p2n = np.asarray(inputs["pose2nlpid"]).astype(np.int64)
    if (not np.array_equal(n2p, np.arange(NN) // K)
            or not np.array_equal(p2n, np.arange(NN).reshape(NP, K))):
        return _kernel_host_fallback(inputs)

    # ---- host: norms, X^T (bf16), poseFT (bf16 + fp8) --------------------
    norm_p = np.sqrt(np.einsum("cpd,cpd->cp", pose, pose, dtype=np.float32,
                               optimize=True)).astype(np.float32)       # [C, NP]
    norm_n = np.sqrt(np.einsum("cnd,cnd->cn", nlp, nlp, dtype=np.float32,
                               optimize=True)).astype(np.float32)       # [C, NN]
    loss_norm = np.float32(np.float32(norm_p.mean()) + np.float32(norm_n.mean()))

    poseF = pose / norm_p[:, :, None]
    poseFT = np.ascontiguousarray(
        poseF.transpose(0, 2, 1).reshape(CD, NP))                       # [CD, NP]
    P8 = (poseFT * PM_SCALE).astype(FP8)                                # [CD, NP]

    scale = (cat.T / norm_n).astype(np.float32)                         # [C, NN]
    X8 = ((nlp * scale[:, :, None]).transpose(0, 2, 1).reshape(CD, NN)
          * X_SCALE).astype(FP8)                                        # [CD, NN]

    # (k, pose)-major column order within each core's 1024 nlp columns
    base_cols = (np.arange(NPL)[None, :] * K + np.arange(K)[:, None]).reshape(-1)

    mskv = np.zeros((128, 2, 256), FP8)
    jj = np.arange(128)
    mskv[jj, 0, jj] = 1
    mskv[jj, 1, jj + 128] = 1

    # ---- device kernel 1: scores + pm blocks -----------------------------
    run1 = _get_runner("k1")
    po_dev = [_po_dev(np.ascontiguousarray(P8[:, c * NPL:(c + 1) * NPL]), mskv)
              for c in range(NCORES)]
    in_maps = []
    for c in range(NCORES):
        i, j = c // PM_NJ, c % PM_NJ
        in_maps.append({
            "x8": _x_dev(X8, c * NNL + base_cols),
            "po8x": po_dev[c],
            "pml": np.ascontiguousarray(P8[:, i * PM_M:(i + 1) * PM_M]),
            "pmr": np.ascontiguousarray(P8[:, j * PM_N:(j + 1) * PM_N]),
        })
    res1 = run1(in_maps)

    scores = _scores_from_dev(res1)                                     # [NN]
    pm = np.empty((NP, NP), np.float32)
    for c in range(NCORES):
        i, j = c // PM_NJ, c % PM_NJ
        pm[i * PM_M:(i + 1) * PM_M, j * PM_N:(j + 1) * PM_N] = \
            res1[c]["pmblk"].astype(np.float32)

    # ---- host: BCE -------------------------------------------------------
    p = (1.0 / (1.0 + np.exp(-scores))).astype(np.float32)
    lblf = nlab.astype(np.float32)
    loss_label = np.float32(
        np.mean(-(np.log(p) * lblf + np.log(1.0 - p) * (1.0 - lblf))))

    # ---- host: furthest selection (pm is PM_SCALE^2 * gram) --------------
    ar = np.arange(NP)
    pm[ar, ar] = PM_SCALE * PM_SCALE
    order = np.argsort(pm, axis=1, kind="stable")
    furthest = order[ar, ri]                                            # [NP]

    sg = scores.reshape(NP, K)
    lg = nlab.reshape(NP, K)
    maxp = np.maximum(np.max(np.where(lg == 0, sg, -np.inf), axis=1), -1.0)
    minp = np.minimum(np.min(np.where(lg == 1, sg, np.inf), axis=1), 1.0)

    # ---- device kernel 2: dots of gathered hard-positive columns ---------
    run2 = _get_runner("k2")
    in_maps2 = []
    for c in range(NCORES):
        # col j = k*NPL + p_local -> global nlp id 4*furthest[p] + k
        fth = furthest[c * NPL:(c + 1) * NPL]
        cols = (np.arange(K)[:, None] + fth[None, :] * K).reshape(-1)
        in_maps2.append({
            "x8": _x_dev(X8, cols),
            "po8x": po_dev[c],
        })
    res2 = run2(in_maps2)
    cur = _scores_from_dev(res2).reshape(NP, K)

    nids = (furthest[:, None] * K + np.arange(K)).reshape(-1)           # [NN]
    lcur = nlab[nids].reshape(NP, K)
    maxcur = np.max(np.where(lcur == 1, cur, -np.inf), axis=1)
    maxp = np.maximum(maxp, maxcur)
    found = ~((maxp == -1.0) | (minp == 1.0))
    lt = np.where(found, maxp - minp + 2.0, 0.0).astype(np.float32)
    not_find = int(np.sum(~found))
    if not_find == NN:
        loss_triple = np.float32(0.0)
    else:
        loss_triple = np.float32(lt.sum(dtype=np.float32) / np.float32(NN - not_find))

    return (np.float32(loss_label), np.float32(loss_norm), np.float32(loss_triple))


# revision 30
# speedup vs baseline: 3.0724x; 1.0051x over previous
"""Trainium2 Bass kernel for nn_ContrastLoss_Disentangle.

Contract: kernel(**inputs) takes the FULL (unsharded) inputs and returns
(loss_label, loss_norm, loss_triple) as float32 scalars.

Pipeline (8 NeuronCores, data-parallel over pose/nlp rows):
  host:    norms (pose+nlp), normalization, X = nlpF * categories / norm
           (bf16, cd-partition interleaved layout), poseFT fp8 (x16) for the
           gram matrix, poseFT bf16 interleaved for the dot products
  device1: per-core: scores for the core's 1024 nlp rows (DVE bf16 mult +
           PE ones-matmul partition reduction) and a [512, 1024] block of
           the pose gram matrix (PE, fp8 DoubleRow), emitted as f16
  host:    BCE, pm assembly + stable argsort rank-select (furthest), gather
           of the hard-positive X columns
  device2: per-core: cur dots of the gathered X columns (same structure)
  host:    triplet loss assembly
"""

import os
import numpy as np
import ml_dtypes

import concourse.bass as bass
import concourse.tile as tile
from concourse import bacc, mybir
from concourse.bass2jax import install_neuronx_cc_hook, partition_id_tensor, _bass_exec_p

C, NP, K, D = 8, 2048, 4, 256
NN = NP * K          # 8192
NCORES = 8
NPL = NP // NCORES   # 256 poses per core
NNL = NN * 1 // NCORES  # 1024 nlp rows per core
CD = C * D           # 2048 contraction size
KC = CD // 128       # 16 cd chunks of 128 partitions

# pm symmetric-cover blocks: each core computes A^T B for its strip sets
# (128-column strips of poseFT); host mirrors into the full gram matrix.
PM_MI = 4            # A strips per core (512 rows)
PM_NI = 6            # B strips per core (768 cols)
PM_M = 128 * PM_MI
PM_N = 128 * PM_NI
PM_NN = PM_N // 2    # psum-bank-sized col chunk (384 f32 < 512)
KT2 = CD // 256      # 8 DoubleRow contraction chunks
# biclique cover of all 136 unordered strip pairs (verified complete)
PM_A = [[0, 1, 2, 3], [4, 5, 6, 7], [8, 9, 10, 11], [12, 13, 14, 15],
        [0, 1, 2, 3], [4, 5, 6, 7], [2, 3, 6, 7], [10, 11, 14, 15]]
PM_B = [[4, 5, 6, 7, 0, 1], [8, 9, 10, 11, 4, 5], [12, 13, 14, 15, 8, 9],
        [0, 1, 2, 3, 12, 13], [8, 9, 10, 11, 0, 1], [12, 13, 14, 15, 4, 5],
        [2, 3, 6, 7, 0, 1], [10, 11, 14, 15, 8, 9]]

PM_SCALE = 16.0      # fp8 pre-scale for pose columns (argsort is scale-invariant)
X_SCALE = 8.0        # fp8 pre-scale for X columns (scores come back x128)

BF16 = ml_dtypes.bfloat16
FP8 = ml_dtypes.float8_e4m3

_runners = {}


NM = 8  # m-chunks of 128 score columns per core


def _build_kernel(with_pm: bool):
    """Per-core program. Inputs (per core):
      x8  [128, NM, KT2, 2, 128] fp8  X columns (x8 scale), m-packed:
          x8[p, m, kc2, t, jj] = X8[kc2*256+t*128+p, col(m*128+jj)]
          with col() the core's (k, pose)-major order
      po8 [128, KT2, 2, NPL] fp8  pose columns (x16 scale), same packing
      msk [128, 2, 256] bf16  diagonal masks: msk[jj, par, q] = (q==jj+128*par)
      pml [CD, PM_M] fp8, pmr [CD, PM_N] fp8  (only when with_pm)
    Outputs:
      sc  [128, NM] f32   128*score[m*128+jj] at [jj, m]
      pmblk [PM_M, PM_N] f16  (only when with_pm)
    """
    f8 = mybir.dt.float8e4
    f16 = mybir.dt.float16
    f32 = mybir.dt.float32
    nc = bacc.Bacc("TRN2", target_bir_lowering=False, debug=False,
                   num_devices=NCORES)
    x8 = nc.dram_tensor("x8", [128, NM, KT2, 2, 128], f8,
                        kind="ExternalInput").ap()
    # po8x: slots 0..KT2-1 = pose columns; slot KT2 = diagonal masks
    po8x = nc.dram_tensor("po8x", [128, KT2 + 1, 2, NPL], f8,
                          kind="ExternalInput").ap()
    if with_pm:
        pml = nc.dram_tensor("pml", [CD, PM_M], f8, kind="ExternalInput").ap()
        pmr = nc.dram_tensor("pmr", [CD, PM_N], f8, kind="ExternalInput").ap()
        pmblk = nc.dram_tensor("pmblk", [PM_M, PM_N], f16,
                               kind="ExternalOutput").ap()
    sc = nc.dram_tensor("sc", [128, NM], f16, kind="ExternalOutput").ap()

    DR = mybir.MatmulPerfMode.DoubleRow
    with tile.TileContext(nc) as tc:
        with tc.tile_pool(name="cst", bufs=1) as cst, \
             tc.tile_pool(name="zp", bufs=3) as zp, \
             tc.tile_pool(name="ev", bufs=2) as ev, \
             tc.tile_pool(name="ps", bufs=2, space="PSUM") as ps, \
             tc.tile_pool(name="ps_s", bufs=2, space="PSUM") as ps_s:

            po8_t = cst.tile([128, KT2 + 1, 2, NPL], f8, tag="po8")
            x8_t = cst.tile([128, NM, KT2, 2, 128], f8, tag="x8")
            sct = cst.tile([128, NM], f16, tag="sct")

            def po_load():
                nc.sync.dma_start(po8_t[:], po8x)

            def x_load(ci, nch=1):
                nc.sync.dma_start(x8_t[:, ci:ci + nch], x8[:, ci:ci + nch])

            def scores_m(m):
                psc = ps_s.tile([128, 256], f32, tag="psc", name=f"psc{m}")
                for kc2 in range(KT2):
                    nc.tensor.matmul(psc[:], x8_t[:, m, kc2],
                                     po8_t[:, kc2][:],
                                     start=(kc2 == 0), stop=(kc2 == KT2 - 1),
                                     perf_mode=DR, skip_group_check=True)
                zm = zp.tile([128, 256], f16, tag="zm", name=f"zm{m}")
                with nc.allow_low_precision(reason="f16 scores ok"):
                    mode = os.environ.get("EXTRACT", "stt")
                    if mode == "2op":
                        nc.vector.tensor_tensor(zm[:], psc[:],
                                                po8_t[:, KT2, m & 1],
                                                op=mybir.AluOpType.mult)
                        nc.vector.tensor_reduce(sct[:, m:m + 1], zm[:],
                                                axis=mybir.AxisListType.X,
                                                op=mybir.AluOpType.add)
                    elif mode == "stt":
                        nc.vector.scalar_tensor_tensor(
                            zm[:], psc[:], 1.0, po8_t[:, KT2, m & 1],
                            op0=mybir.AluOpType.mult,
                            op1=mybir.AluOpType.mult,
                            accum_out=sct[:, m:m + 1])
                    else:
                        nc.vector.tensor_tensor_reduce(
                            zm[:], psc[:], po8_t[:, KT2, m & 1], scale=1.0,
                            scalar=0.0, op0=mybir.AluOpType.mult,
                            op1=mybir.AluOpType.add, accum_out=sct[:, m:m + 1])

            def s_flush():
                nc.scalar.dma_start(sc, sct[:])

            # ---------- pm sub-program ------------------------------------
            if with_pm:
                lt = cst.tile([128, KT2, 2, PM_M], f8, tag="lt")
                rt = cst.tile([128, KT2, 2, PM_N], f8, tag="rt")

                def lt_load(half):
                    nc.sync.dma_start(
                        lt[:, 4 * half:4 * (half + 1)],
                        pml[1024 * half:1024 * (half + 1), :]
                        .rearrange("(kc t p) m -> p kc t m", p=128, t=2))

                def rt_load(half):
                    nc.sync.dma_start(
                        rt[:, 4 * half:4 * (half + 1)],
                        pmr.rearrange("(kc t p) q -> p kc t q", p=128, t=2)
                        [:, 4 * half:4 * (half + 1)])

                evs = {}

                def pm_mchain(n, m):
                    acc = ps.tile([128, PM_NN], f32, tag="acc",
                                  name=f"acc{n}{m}")
                    for kc in range(KT2):
                        nc.tensor.matmul(acc[:],
                                         lt[:, kc, :, 128 * m:128 * (m + 1)],
                                         rt[:, kc, :,
                                            PM_NN * n:PM_NN * (n + 1)],
                                         start=(kc == 0),
                                         stop=(kc == KT2 - 1),
                                         perf_mode=DR, skip_group_check=True)
                    if m == 0:
                        evs[n] = ev.tile([128, 4, PM_NN], f16, tag="ev",
                                         name=f"ev{n}")
                    nc.scalar.copy(evs[n][:, m][:], acc[:])
                    if m == PM_MI - 1:
                        nc.scalar.dma_start(
                            pmblk.rearrange("(m p) q -> p m q", p=128)
                            [:, :, PM_NN * n:PM_NN * (n + 1)], evs[n][:])

            # ---------- emission order ------------------------------------
            if with_pm:
                lt_load(0)
                rt_load(0)
                lt_load(1)
                rt_load(1)
                pm_mchain(0, 0)
                pm_mchain(0, 1)
                po_load()
                pm_mchain(0, 2)
                x_load(0, 2)
                pm_mchain(0, 3)
                pm_mchain(1, 0)
                x_load(2, 2)
                pm_mchain(1, 1)
                scores_m(0)
                pm_mchain(1, 2)
                scores_m(1)
                x_load(4, 2)
                pm_mchain(1, 3)
                scores_m(2)
                scores_m(3)
                x_load(6, 2)
                scores_m(4)
                scores_m(5)
                scores_m(6)
                scores_m(7)
                s_flush()
            else:
                po_load()
                for m in range(NM):
                    x_load(m)
                    scores_m(m)
                s_flush()

    nc.finalize()
    return nc


def _make_runner(nc):
    """Reusable jitted SPMD runner (replicates bass2jax.run_bass_via_pjrt but
    caches the compiled executable across calls)."""
    import jax
    from jax.sharding import Mesh, PartitionSpec
    from jax.experimental.shard_map import shard_map

    install_neuronx_cc_hook()
    partition_name = nc.partition_id_tensor.name if nc.partition_id_tensor else None
    in_names, out_names, out_avals = [], [], []
    for alloc in nc.m.functions[0].allocations:
        if not isinstance(alloc, mybir.MemoryLocationSet):
            continue
        name = alloc.memorylocations[0].name
        if alloc.kind == "ExternalInput":
            if name != partition_name:
                in_names.append(name)
        elif alloc.kind == "ExternalOutput":
            out_names.append(name)
            out_avals.append(jax.core.ShapedArray(
                tuple(alloc.tensor_shape), mybir.dt.np(alloc.dtype)))
    n_params = len(in_names)
    all_in = in_names + out_names + ([partition_name] if partition_name else [])

    def _body(*args):
        operands = list(args)
        if partition_name is not None:
            operands.append(partition_id_tensor())
        outs = _bass_exec_p.bind(
            *operands, out_avals=tuple(out_avals), in_names=tuple(all_in),
            out_names=tuple(out_names), lowering_input_output_aliases=(),
            sim_require_finite=False, sim_require_nnan=False, nc=nc)
        return tuple(outs)

    devices = jax.devices()[:NCORES]
    mesh = Mesh(np.asarray(devices), ("core",))
    donate = tuple(range(n_params, n_params + len(out_names)))
    sharded = jax.jit(
        shard_map(_body, mesh=mesh,
                  in_specs=(PartitionSpec("core"),) * (n_params + len(out_names)),
                  out_specs=(PartitionSpec("core"),) * len(out_names),
                  check_rep=False),
        donate_argnums=donate, keep_unused=True)

    def run(in_maps):
        concat_in = [np.concatenate([np.asarray(m[name]) for m in in_maps], axis=0)
                     for name in in_names]
        zeros = [np.zeros((NCORES * a.shape[0], *a.shape[1:]), a.dtype)
                 for a in out_avals]
        out_arrs = sharded(*concat_in, *zeros)
        return [
            {name: np.asarray(out_arrs[i]).reshape(NCORES, *out_avals[i].shape)[c]
             for i, name in enumerate(out_names)}
            for c in range(NCORES)
        ]

    return run


def _get_runner(key):
    if key not in _runners:
        _runners[key] = _make_runner(_build_kernel(with_pm=(key == "k1")))
    return _runners[key]


def _x_dev(X8, cols):
    """Columns `cols` (global nlp ids, (k, pose)-major order per core) of the
    fp8 [CD, NN] X^T matrix -> [128, NM, KT2, 2, 128] device layout."""
    A = X8[:, cols]                                # [CD, 1024]
    return np.ascontiguousarray(
        A.reshape(KT2, 2, 128, NM, 128).transpose(2, 3, 0, 1, 4))


def _po_dev(P8c, mskv):
    """[CD, NPL] fp8 pose columns -> [128, KT2+1, 2, NPL] device layout with
    the diagonal masks appended as slot KT2."""
    po = P8c.reshape(KT2, 2, 128, NPL).transpose(2, 0, 1, 3)
    return np.ascontiguousarray(
        np.concatenate([po, mskv[:, None, :, :]], axis=1))


def _scores_from_dev(res, name="sc"):
    """Per-core [128, NM] (row jj, col m -> j = m*128+jj, scaled x128, with
    col j = k*NPL + p_local) -> [NN] global scores."""
    out = np.empty((NCORES, NPL, K), np.float32)
    for c in range(NCORES):
        sc = np.asarray(res[c][name], np.float32)
        out[c] = (sc.T.reshape(K * NPL) / 128.0).reshape(K, NPL).T
    return out.reshape(NN)


def _kernel_host_fallback(inputs):
    """Pure-numpy reference replication, used only if the index tensors do
    not have the canonical arange structure the device layout relies on."""
    nlp = np.asarray(inputs["nlp_features"], np.float32)
    pose = np.asarray(inputs["pose_features"], np.float32)
    nlab = np.asarray(inputs["nlp_label"]).astype(np.int64)
    n2p = np.asarray(inputs["nlpid2poseid"]).astype(np.int64)
    p2n = np.asarray(inputs["pose2nlpid"]).astype(np.int64)
    cat = np.asarray(inputs["categories"], np.float32)
    ri = np.asarray(inputs["rand_index"]).astype(np.int64)
    Np, Nn = pose.shape[1], nlp.shape[1]
    norm_p = np.sqrt(np.einsum("cpd,cpd->cp", pose, pose, dtype=np.float32))
    norm_n = np.sqrt(np.einsum("cnd,cnd->cn", nlp, nlp, dtype=np.float32))
    poseF = pose / norm_p[:, :, None]
    nlpF = nlp / norm_n[:, :, None]
    loss_norm = np.float32(np.float32(norm_p.mean()) + np.float32(norm_n.mean()))
    dots = np.einsum("cnd,cnd->cn", nlpF, poseF[:, n2p]).astype(np.float32)
    scores = np.einsum("cn,nc->n", dots, cat).astype(np.float32)
    p = (1.0 / (1.0 + np.exp(-scores))).astype(np.float32)
    lblf = nlab.astype(np.float32)
    loss_label = np.float32(
        np.mean(-(np.log(p) * lblf + np.log(1.0 - p) * (1.0 - lblf))))
    pf = np.ascontiguousarray(poseF.transpose(0, 2, 1).reshape(-1, Np))
    pm = (pf.T @ pf).astype(np.float32)
    ar = np.arange(Np)
    pm[ar, ar] = 1.0
    order = np.argsort(pm, axis=1, kind="stable")
    furthest = order[ar, ri]
    sg = scores[p2n]
    lg = nlab[p2n]
    maxp = np.maximum(np.max(np.where(lg == 0, sg, -np.inf), axis=1), -1.0)
    minp = np.minimum(np.min(np.where(lg == 1, sg, np.inf), axis=1), 1.0)
    nids = p2n[furthest]
    cd = np.einsum("cpkd,cpd->cpk", nlpF[:, nids], poseF)
    cur = np.einsum("cpk,pkc->pk", cd, cat[nids]).astype(np.float32)
    lcur = nlab[nids]
    maxcur = np.max(np.where(lcur == 1, cur, -np.inf), axis=1)
    maxp = np.maximum(maxp, maxcur)
    found = ~((maxp == -1.0) | (minp == 1.0))
    lt = np.where(found, maxp - minp + 2.0, 0.0).astype(np.float32)
    nf = int(np.sum(~found))
    loss_triple = (np.float32(0.0) if nf == Nn else
                   np.float32(lt.sum(dtype=np.float32) / np.float32(Nn - nf)))
    return (np.float32(loss_label), loss_norm, loss_triple)


def kernel(**inputs):
    nlp = np.ascontiguousarray(inputs["nlp_features"], np.float32)      # [C, NN, D]
    pose = np.ascontiguousarray(inputs["pose_features"], np.float32)    # [C, NP, D]
    nlab = np.asarray(inputs["nlp_label"]).astype(np.int64)
    cat = np.ascontiguousarray(inputs["categories"], np.float32)        # [NN, C]
    ri = np.asarray(inputs["rand_index"]).astype(np.int64)

    n2p = np.asarray(inputs["nlpid2poseid"]).astype(np.int64)
    p2n = np.asarray(inputs["pose2nlpid"]).astype(np.int64)
    if (not np.array_equal(n2p, np.arange(NN) // K)
            or not np.array_equal(p2n, np.arange(NN).reshape(NP, K))):
        return _kernel_host_fallback(inputs)

    # ---- host: norms, X^T (bf16), poseFT (bf16 + fp8) --------------------
    norm_p = np.sqrt(np.einsum("cpd,cpd->cp", pose, pose, dtype=np.float32,
                               optimize=True)).astype(np.float32)       # [C, NP]
    norm_n = np.sqrt(np.einsum("cnd,cnd->cn", nlp, nlp, dtype=np.float32,
                               optimize=True)).astype(np.float32)       # [C, NN]
    loss_norm = np.float32(np.float32(norm_p.mean()) + np.float32(norm_n.mean()))

    poseF = pose / norm_p[:, :, None]
    poseFT = np.ascontiguousarray(
        poseF.transpose(0, 2, 1).reshape(CD, NP))                       # [CD, NP]
    P8 = (poseFT * PM_SCALE).astype(FP8)                                # [CD, NP]

    scale = (cat.T / norm_n).astype(np.float32)                         # [C, NN]
    X8 = ((nlp * scale[:, :, None]).transpose(0, 2, 1).reshape(CD, NN)
          * X_SCALE).astype(FP8)                                        # [CD, NN]

    # (k, pose)-major column order within each core's 1024 nlp columns
    base_cols = (np.arange(NPL)[None, :] * K + np.arange(K)[:, None]).reshape(-1)

    mskv = np.zeros((128, 2, 256), FP8)
    jj = np.arange(128)
    mskv[jj, 0, jj] = 1
    mskv[jj, 1, jj + 128] = 1

    # ---- device kernel 1: scores + pm blocks -----------------------------
    run1 = _get_runner("k1")
    po_dev = [_po_dev(np.ascontiguousarray(P8[:, c * NPL:(c + 1) * NPL]), mskv)
              for c in range(NCORES)]
    def strip_cols(strips):
        return np.concatenate(
            [np.arange(128 * s, 128 * (s + 1)) for s in strips])

    in_maps = []
    for c in range(NCORES):
        in_maps.append({
            "x8": _x_dev(X8, c * NNL + base_cols),
            "po8x": po_dev[c],
            "pml": np.ascontiguousarray(P8[:, strip_cols(PM_A[c])]),
            "pmr": np.ascontiguousarray(P8[:, strip_cols(PM_B[c])]),
        })
    res1 = run1(in_maps)

    scores = _scores_from_dev(res1)                                     # [NN]
    pm = np.empty((NP, NP), np.float32)
    for c in range(NCORES):
        blk = np.asarray(res1[c]["pmblk"]).astype(np.float32)           # [512, 768]
        for ai, sa in enumerate(PM_A[c]):
            for bi, sb in enumerate(PM_B[c]):
                sub = blk[128 * ai:128 * (ai + 1), 128 * bi:128 * (bi + 1)]
                pm[128 * sa:128 * (sa + 1), 128 * sb:128 * (sb + 1)] = sub
                pm[128 * sb:128 * (sb + 1), 128 * sa:128 * (sa + 1)] = sub.T

    # ---- host: BCE -------------------------------------------------------
    p = (1.0 / (1.0 + np.exp(-scores))).astype(np.float32)
    lblf = nlab.astype(np.float32)
    loss_label = np.float32(
        np.mean(-(np.log(p) * lblf + np.log(1.0 - p) * (1.0 - lblf))))

    # ---- host: furthest selection (pm is PM_SCALE^2 * gram) --------------
    ar = np.arange(NP)
    pm[ar, ar] = PM_SCALE * PM_SCALE
    order = np.argsort(pm, axis=1, kind="stable")
    furthest = order[ar, ri]                                            # [NP]

    sg = scores.reshape(NP, K)
    lg = nlab.reshape(NP, K)
    maxp = np.maximum(np.max(np.where(lg == 0, sg, -np.inf), axis=1), -1.0)
    minp = np.minimum(np.min(np.where(lg == 1, sg, np.inf), axis=1), 1.0)

    # ---- device kernel 2: dots of gathered hard-positive columns ---------
    run2 = _get_runner("k2")
    in_maps2 = []
    for c in range(NCORES):
        # col j = k*NPL + p_local -> global nlp id 4*furthest[p] + k
        fth = furthest[c * NPL:(c + 1) * NPL]
        cols = (np.arange(K)[:, None] + fth[None, :] * K).reshape(-1)
        in_maps2.append({
            "x8": _x_dev(X8, cols),
            "po8x": po_dev[c],
        })
    res2 = run2(in_maps2)
    cur = _scores_from_dev(res2).reshape(NP, K)

    nids = (furthest[:, None] * K + np.arange(K)).reshape(-1)           # [NN]
    lcur = nlab[nids].reshape(NP, K)
    maxcur = np.max(np.where(lcur == 1, cur, -np.inf), axis=1)
    maxp = np.maximum(maxp, maxcur)
    found = ~((maxp == -1.0) | (minp == 1.0))
    lt = np.where(found, maxp - minp + 2.0, 0.0).astype(np.float32)
    not_find = int(np.sum(~found))
    if not_find == NN:
        loss_triple = np.float32(0.0)
    else:
        loss_triple = np.float32(lt.sum(dtype=np.float32) / np.float32(NN - not_find))

    return (np.float32(loss_label), np.float32(loss_norm), np.float32(loss_triple))
